# revision 1
# baseline (speedup 1.0000x reference)
"""Trainium2 Bass kernel for the differentiable EXP-HYDRO module.

Strategy (8 NeuronCores, data-parallel over the catchment axis):
  - Each core gets 16 catchments x 4096 timesteps.
  - Parameterization MLP runs on the PE (fp32 matmuls, hidden-major layout),
    tanh/sigmoid on the ACT engine (sigmoid(x) = (tanh(x/2)+1)/2, folded into
    the downstream affine transforms so only the exp_and_others table is used).
  - The sequential bucket scan is solved parallel-in-time: each state's
    trajectory satisfies S[t] = F(S[t-1], t).  We iterate
        r_t = F(Sprev_t, t) - S_t
        delta_t = J_t * delta_{t-1} + r_t     (hardware tensor_tensor_scan)
        S += delta
    with a stable propagator J (frozen-gate for the snow bucket, clamped
    Newton for the soil bucket).  The fixed point is the exact fp32
    recurrence regardless of J.  Layout: [128 partitions = 16 catchments x 8
    time-blocks, 512 steps]; block-boundary carries are stitched with a
    Kogge-Stone pass over partitions using PE shift matrices.
  - 31 snow sweeps + 3 soil sweeps converge to ~1e-5 of the reference.
"""

import os
import numpy as np
from contextlib import ExitStack

import concourse.bass as bass
import concourse.bacc as bacc
import concourse.mybir as mybir
import concourse.tile as tile
from concourse import bass_utils

F32 = mybir.dt.float32
F32R = mybir.dt.float32r
Op = mybir.AluOpType
Act = mybir.ActivationFunctionType

B, T, NF = 128, 4096, 20
NCORES = 8
BC = B // NCORES          # catchments per core = 16
NB = 8                    # time blocks per catchment
L = T // NB               # 512 steps per block
PP = BC * NB              # 128 partitions
N_S0 = 31                 # snow-bucket sweeps
N_S1 = 3                  # soil-bucket sweeps
H1, H2 = 256, 64


def _host_constants():
    """Kogge-Stone shift matrices (partition-space, catchment-masked) and
    the fill columns for the multiplicative combine."""
    ks = np.zeros((3, PP, PP), np.float32)
    zc = np.zeros((PP, 3), np.float32)
    for ki, k in enumerate((1, 2, 4)):
        for mcol in range(PP):
            if (mcol % NB) >= k:
                ks[ki, mcol - k, mcol] = 1.0
        zc[:, ki] = (np.arange(PP) % NB < k).astype(np.float32)
    return ks, zc


def _build_kernel(tc, outs, ins):
    nc = tc.nc
    (att, met, w1k, b1, w2r, w2s, b2, w3, b3, ksm, zcm) = ins
    q_out = outs[0]

    with ExitStack() as ctx:
        const = ctx.enter_context(tc.tile_pool(name="const", bufs=1))
        spool = ctx.enter_context(tc.tile_pool(name="scan", bufs=1))
        dpool = ctx.enter_context(tc.tile_pool(name="dram", bufs=1, space="DRAM"))

        # ---- constants ----
        w1ks = const.tile([60, H1], F32R)
        nc.sync.dma_start(w1ks[:], w1k[:])
        w2ar = const.tile([128, H2], F32R)
        nc.sync.dma_start(w2ar[:], w2r[0:128, :])
        w2br = const.tile([128, H2], F32R)
        nc.sync.dma_start(w2br[:], w2r[128:256, :])
        w2as = const.tile([128, H2], F32R)
        nc.sync.dma_start(w2as[:], w2s[0:128, :])
        w2bs = const.tile([128, H2], F32R)
        nc.sync.dma_start(w2bs[:], w2s[128:256, :])
        # w3 extended with the b3 row; the matching lhsT ones-row folds the
        # bias into the matmul exactly.
        w3e = const.tile([H2 + 1, 6], F32)
        nc.sync.dma_start(w3e[0:H2, :], w3[:])
        nc.sync.dma_start(w3e[H2 : H2 + 1, :], b3.rearrange("(o p) -> o p", o=1))
        b1s = const.tile([128, 2], F32)
        nc.sync.dma_start(b1s[:], b1.rearrange("(h p) -> p h", p=128))
        b2s = const.tile([H2, 1], F32)
        nc.sync.dma_start(b2s[:], b2.rearrange("(p o) -> p o", o=1))
        # double-buffered extended-h2 tiles with a constant ones row
        h2e_a = const.tile([H2 + 1, 1024], F32)
        h2e_b = const.tile([H2 + 1, 1024], F32)
        nc.vector.memset(h2e_a[H2 : H2 + 1, :], 1.0)
        nc.vector.memset(h2e_b[H2 : H2 + 1, :], 1.0)
        ks1 = const.tile([PP, PP], F32)
        nc.sync.dma_start(ks1[:], ksm[0])
        ks2 = const.tile([PP, PP], F32)
        nc.sync.dma_start(ks2[:], ksm[1])
        ks4 = const.tile([PP, PP], F32)
        nc.sync.dma_start(ks4[:], ksm[2])
        zc = const.tile([PP, 3], F32)
        nc.sync.dma_start(zc[:], zcm[:])
        ones = const.tile([PP, L], F32)
        nc.vector.memset(ones[:], 1.0)
        cm75 = const.tile([PP, 1], F32)
        nc.vector.memset(cm75[:], -7.5)

        # ---- DRAM staging ----
        params_d = dpool.tile([PP, 6 * L], F32)

        # ---- MLP phase ----
        # fp32r matmuls (full-rate PE), pair-batched ACT calls, and L3 run
        # tokens-on-M (lhsT = h2 chunks) so its PE+ACT cost is tiny.

        with tc.tile_pool(name="mlp_in", bufs=2) as tpool, \
             tc.tile_pool(name="mlp_ps", bufs=2, space="PSUM") as ppool, \
             tc.tile_pool(name="mlp_h", bufs=2) as hpool:
            for c in range(BC):
                attrs_t = tpool.tile([60, T], F32R, tag="attrs")
                nc.sync.dma_start(attrs_t[:], att[c])
                h1 = {}
                for half in (0, 1):
                    for pq in range(4):  # block pair (2*pq, 2*pq+1)
                        ps1 = ppool.tile([128, 2 * L], F32, tag="l1")
                        hs = slice(half * 128, half * 128 + 128)
                        for bi in (0, 1):
                            bb = 2 * pq + bi
                            ts = slice(bb * L, (bb + 1) * L)
                            nc.tensor.matmul(
                                ps1[:, bi * L : (bi + 1) * L],
                                w1ks[:, hs],
                                attrs_t[:, ts],
                                start=True, stop=True,
                            )
                        ht = hpool.tile(
                            [128, 2 * L], F32, tag=f"h1_{half}_{pq}",
                            name=f"h1_{half}_{pq}", bufs=1,
                        )
                        nc.scalar.activation(
                            ht[:], ps1[:], Act.Tanh, bias=b1s[:, half : half + 1]
                        )
                        htr = hpool.tile(
                            [128, 2 * L], F32R, tag=f"h1r_{half}_{pq}",
                            name=f"h1r_{half}_{pq}", bufs=1,
                        )
                        nc.vector.tensor_copy(htr[:], ht[:])
                        hts = hpool.tile(
                            [128, 2 * L], F32R, tag=f"h1s_{half}_{pq}",
                            name=f"h1s_{half}_{pq}", bufs=1,
                        )
                        nc.vector.tensor_tensor(hts[:], ht[:], htr[:], Op.subtract)
                        h1[(half, pq)] = (htr, hts)
                for pq in range(4):
                    ps2 = ppool.tile([H2, 2 * L], F32, tag="l23", bufs=2, name="ps2")
                    for bi in (0, 1):
                        sl = slice(bi * L, (bi + 1) * L)
                        h0r, h0s = h1[(0, pq)]
                        h1r_, h1s_ = h1[(1, pq)]
                        nc.tensor.matmul(ps2[:, sl], w2ar[:], h0r[:, sl],
                                         start=True, stop=False)
                        nc.tensor.matmul(ps2[:, sl], w2br[:], h1r_[:, sl],
                                         start=False, stop=False)
                        nc.tensor.matmul(ps2[:, sl], w2ar[:], h0s[:, sl],
                                         start=False, stop=False)
                        nc.tensor.matmul(ps2[:, sl], w2br[:], h1s_[:, sl],
                                         start=False, stop=False)
                        nc.tensor.matmul(ps2[:, sl], w2as[:], h0r[:, sl],
                                         start=False, stop=False)
                        nc.tensor.matmul(ps2[:, sl], w2bs[:], h1r_[:, sl],
                                         start=False, stop=True)
                    h2e = h2e_a if pq % 2 == 0 else h2e_b
                    nc.scalar.activation(h2e[0:H2, :], ps2[:], Act.Tanh, bias=b2s[:])
                    ps3 = ppool.tile([128, 48], F32, tag="l23", bufs=2, name="ps3")
                    for bi in (0, 1):
                        for ch in range(4):
                            nc.tensor.matmul(
                                ps3[:, bi * 24 + ch * 6 : bi * 24 + ch * 6 + 6],
                                h2e[:, bi * L + ch * 128 : bi * L + (ch + 1) * 128],
                                w3e[:],
                                start=True, stop=True,
                            )
                    u3 = hpool.tile([128, 48], F32, tag="u3")
                    nc.scalar.activation(u3[:], ps3[:], Act.Tanh, scale=0.5)
                    for bi in (0, 1):
                        p = c * NB + 2 * pq + bi
                        dst = params_d[p : p + 1, :].rearrange(
                            "o (i ch v) -> (o i) ch v", v=6, ch=4, i=128
                        )
                        srcv = u3[:, bi * 24 : (bi + 1) * 24].rearrange(
                            "p (ch v) -> p ch v", ch=4
                        )
                        nc.sync.dma_start(dst, srcv)

        wpool = ctx.enter_context(tc.tile_pool(name="work", bufs=1))

        # ---- gather to scan layout [128, 512] ----
        pall = spool.tile([PP, 6 * L], F32)
        nc.sync.dma_start(pall[:], params_d[:])
        pview = pall.rearrange("p (i ch v) -> p ch i v", i=128, ch=4, v=6)
        U = []
        for v in range(6):
            uv = spool.tile([PP, L], F32, name=f"uparam{v}")
            nc.vector.tensor_copy(
                uv.rearrange("p (ch i) -> p ch i", ch=4), pview[:, :, :, v]
            )
            U.append(uv)
        petT = spool.tile([PP, L], F32)
        nc.sync.dma_start(petT[:], met[0])
        tmT = spool.tile([PP, L], F32)
        nc.sync.dma_start(tmT[:], met[1])
        prT = spool.tile([PP, L], F32)
        nc.sync.dma_start(prT[:], met[2])

        # ---- coefficient precompute ----
        ph = spool.tile([PP, L], F32)
        nc.vector.tensor_scalar_mul(ph[:], prT[:], 0.5)
        wps = wpool.tile([PP, L], F32, tag="dd", name="wps")
        nc.vector.scalar_tensor_tensor(wps[:], U[0][:], -1.5, tmT[:], Op.mult, Op.subtract)
        ups = wpool.tile([PP, L], F32, tag="u0", name="ups")
        nc.scalar.activation(ups[:], wps[:], Act.Tanh, bias=cm75[:], scale=5.0)
        psnow = spool.tile([PP, L], F32)
        nc.vector.scalar_tensor_tensor(psnow[:], ups[:], 1.0, ph[:], Op.add, Op.mult)
        om = wpool.tile([PP, L], F32, tag="u1", name="om")
        nc.vector.tensor_scalar(om[:], ups[:], -1.0, 1.0, Op.mult, Op.add)
        prain = spool.tile([PP, L], F32)
        nc.vector.tensor_mul(prain[:], om[:], ph[:])
        wA = wpool.tile([PP, L], F32, tag="ea", name="wA")
        nc.vector.scalar_tensor_tensor(wA[:], U[1][:], -1.5, tmT[:], Op.mult, Op.add)
        uA = wpool.tile([PP, L], F32, tag="eac", name="uA")
        nc.scalar.activation(uA[:], wA[:], Act.Tanh, bias=cm75[:], scale=5.0)
        Ah2 = spool.tile([PP, L], F32)
        nc.vector.tensor_scalar(Ah2[:], uA[:], 0.25, 0.25, Op.mult, Op.add)
        xm = wpool.tile([PP, L], F32, tag="Ee", name="xm")
        nc.vector.tensor_scalar_add(xm[:], wA[:], -1.5)
        d5 = wpool.tile([PP, L], F32, tag="h1", name="d5")
        nc.vector.tensor_scalar(d5[:], U[2][:], 2.5, 2.5, Op.mult, Op.add)
        mT = spool.tile([PP, L], F32)
        nc.vector.tensor_mul(mT[:], d5[:], xm[:])
        fT = spool.tile([PP, L], F32)
        nc.vector.tensor_scalar(fT[:], U[3][:], 0.05, 0.05, Op.mult, Op.add)
        smaxT = spool.tile([PP, L], F32)
        nc.vector.tensor_scalar(smaxT[:], U[4][:], 700.0, 800.0, Op.mult, Op.add)
        qmaxT = spool.tile([PP, L], F32)
        nc.vector.tensor_scalar(qmaxT[:], U[5][:], 20.0, 30.0, Op.mult, Op.add)
        invs = spool.tile([PP, L], F32)
        nc.vector.reciprocal(invs[:], smaxT[:])
        FQ = spool.tile([PP, L], F32)
        nc.vector.tensor_mul(FQ[:], fT[:], qmaxT[:])

        # ---- state tiles ----
        S0 = spool.tile([PP, L], F32)
        nc.vector.memset(S0[:], 0.0)
        SP0 = spool.tile([PP, L], F32)
        nc.vector.memset(SP0[:], 0.0)
        S1 = spool.tile([PP, L], F32)
        nc.vector.memset(S1[:], 0.0)
        SP1 = spool.tile([PP, L], F32)
        nc.vector.memset(SP1[:], 0.0)
        RT = spool.tile([PP, L], F32)

        with tc.tile_pool(name="ks_ps", bufs=2, space="PSUM") as kpool:

            def boundary_fix(scp):
                """Exclusive block-carry via Kogge-Stone over partitions.
                scp = [dp | gp] side by side; one matmul shifts both."""
                cols = scp.rearrange("p (two l) -> p l two", two=2)[:, L - 1, :]
                p_cur, g_cur = cols[:, 0:1], cols[:, 1:2]
                rhs = cols
                for ki, (k, mat) in enumerate(((1, ks1), (2, ks2), (4, ks4))):
                    psr = kpool.tile([PP, 2], F32, tag="psr", name=f"psr{ki}")
                    nc.tensor.matmul(psr[:], mat[:], rhs, start=True, stop=True)
                    gp_n = wpool.tile([PP, 2], F32, tag=f"gpn{ki}", name=f"gpn{ki}")
                    nc.vector.scalar_tensor_tensor(
                        gp_n[:, 1:2], psr[:, 1:2], zc[:, ki : ki + 1], g_cur,
                        Op.add, Op.mult,
                    )
                    nc.vector.scalar_tensor_tensor(
                        gp_n[:, 0:1], psr[:, 0:1], g_cur, p_cur, Op.mult, Op.add
                    )
                    p_cur, g_cur = gp_n[:, 0:1], gp_n[:, 1:2]
                    rhs = gp_n[:]
                psd = kpool.tile([PP, 1], F32, tag="psd", name="psd")
                nc.tensor.matmul(psd[:], ks1[:], p_cur, start=True, stop=True)
                ds = wpool.tile([PP, 1], F32, tag="ksds", name="ksds")
                nc.vector.tensor_copy(ds[:], psd[:])
                return ds

            def apply_delta(S, SP, scp, pre_s, ds):
                # S_new = (S + dp) + gp*ds ; SPREV_new shifted by one step
                nc.vector.scalar_tensor_tensor(
                    S[:], scp[:, L : 2 * L], ds[:], pre_s[:], Op.mult, Op.add
                )
                nc.vector.scalar_tensor_tensor(
                    SP[:, 1:L], scp[:, L : 2 * L - 1], ds[:], pre_s[:, 0 : L - 1],
                    Op.mult, Op.add,
                )
                nc.vector.tensor_add(SP[:, 0:1], SP[:, 0:1], ds[:])

            def tw(nm):
                return wpool.tile([PP, L], F32, tag=nm, name=nm)

            # ---- snow bucket sweeps (frozen-gate propagator) ----
            for it in range(N_S0):
                u = tw("u0")
                nc.scalar.activation(u[:], SP0[:], Act.Tanh, scale=5.0)
                AH = tw("ab")
                nc.vector.scalar_tensor_tensor(AH[:], u[:], 1.0, Ah2[:], Op.add, Op.mult)
                mn = tw("be")
                nc.vector.tensor_tensor(mn[:], SP0[:], mT[:], Op.min)
                ltf = tw("sv")
                nc.vector.tensor_tensor(ltf[:], SP0[:], mT[:], Op.is_lt)
                melt = tw("e1")
                nc.vector.tensor_mul(melt[:], AH[:], mn[:])
                jt = tw("e2")
                nc.vector.tensor_mul(jt[:], AH[:], ltf[:])
                Jt = tw("s1J")
                nc.vector.tensor_scalar(Jt[:], jt[:], -1.0, 1.0, Op.mult, Op.add)
                t1 = tw("e3")
                nc.vector.tensor_sub(t1[:], psnow[:], melt[:])
                t2 = tw("t2b")
                nc.vector.tensor_sub(t2[:], SP0[:], S0[:])
                rr = tw("s1r")
                nc.vector.tensor_add(rr[:], t1[:], t2[:])
                scp = wpool.tile([PP, 2 * L], F32, tag="scp", name="scp")
                nc.vector.tensor_tensor_scan(
                    scp[:, L : 2 * L], Jt[:], ones[:], 1.0, Op.mult, Op.mult
                )
                nc.vector.tensor_tensor_scan(
                    scp[:, 0:L], Jt[:], rr[:], 0.0, Op.mult, Op.add
                )
                pre_s = tw("pre_s")
                nc.vector.tensor_add(pre_s[:], S0[:], scp[:, 0:L])
                ds = boundary_fix(scp)
                apply_delta(S0, SP0, scp, pre_s, ds)

            # ---- melt from converged snow state, rain+melt forcing ----
            u = tw("u0")
            nc.scalar.activation(u[:], SP0[:], Act.Tanh, scale=5.0)
            AH = tw("ab")
            nc.vector.scalar_tensor_tensor(AH[:], u[:], 1.0, Ah2[:], Op.add, Op.mult)
            mn = tw("be")
            nc.vector.tensor_tensor(mn[:], SP0[:], mT[:], Op.min)
            melt = tw("e1")
            nc.vector.tensor_mul(melt[:], AH[:], mn[:])
            nc.vector.tensor_add(RT[:], prain[:], melt[:])

            # ---- soil bucket sweeps (clamped Newton propagator) ----
            for it in range(N_S1):
                u0 = tw("u0")
                nc.scalar.activation(u0[:], SP1[:], Act.Tanh, scale=5.0)
                dd = tw("dd")
                nc.vector.tensor_sub(dd[:], SP1[:], smaxT[:])
                u1 = tw("u1")
                nc.scalar.activation(u1[:], dd[:], Act.Tanh, scale=5.0)
                ea = tw("ea")
                nc.vector.tensor_mul(ea[:], fT[:], dd[:])
                eac = tw("eac")
                nc.vector.tensor_scalar_min(eac[:], ea[:], 2.0)
                Ee = tw("Ee")
                nc.scalar.activation(Ee[:], eac[:], Act.Exp)
                h1 = tw("h1")
                nc.vector.tensor_scalar(h1[:], u0[:], 0.5, 0.5, Op.mult, Op.add)
                ab = tw("ab")
                nc.vector.tensor_scalar(ab[:], u1[:], 0.5, 0.5, Op.mult, Op.add)
                be = tw("be")
                nc.vector.tensor_scalar(be[:], u1[:], -0.5, 0.5, Op.mult, Op.add)
                sv = tw("sv")
                nc.gpsimd.tensor_tensor(sv[:], SP1[:], invs[:], Op.mult)
                e1 = tw("e1")
                nc.vector.tensor_mul(e1[:], be[:], sv[:])
                e2 = tw("e2")
                nc.vector.tensor_add(e2[:], ab[:], e1[:])
                e3 = tw("e3")
                nc.vector.tensor_mul(e3[:], petT[:], e2[:])
                et = tw("et")
                nc.vector.tensor_mul(et[:], h1[:], e3[:])
                q1 = tw("q1")
                nc.vector.tensor_mul(q1[:], be[:], Ee[:])
                q2 = tw("q2")
                nc.vector.tensor_add(q2[:], ab[:], q1[:])
                q3 = tw("q3")
                nc.vector.tensor_mul(q3[:], qmaxT[:], q2[:])
                qsub = tw("qsub")
                nc.vector.tensor_mul(qsub[:], h1[:], q3[:])
                s1a = tw("s1a")
                nc.vector.tensor_mul(s1a[:], h1[:], ab[:])
                qsurf = tw("qsurf")
                nc.vector.tensor_mul(qsurf[:], s1a[:], dd[:])
                g1 = tw("g1")
                nc.vector.tensor_sub(g1[:], RT[:], et[:])
                g2 = tw("g2")
                nc.vector.tensor_sub(g2[:], g1[:], qsub[:])
                gg = tw("gg")
                nc.vector.tensor_sub(gg[:], g2[:], qsurf[:])
                t2b = tw("t2b")
                nc.gpsimd.tensor_tensor(t2b[:], SP1[:], S1[:], Op.subtract)
                rr = tw("s1r")
                nc.vector.tensor_add(rr[:], t2b[:], gg[:])
                u0sq = tw("u0sq")
                nc.gpsimd.tensor_tensor(u0sq[:], u0[:], u0[:], Op.mult)
                h1p = tw("h1p")
                nc.vector.tensor_scalar(h1p[:], u0sq[:], -2.5, 2.5, Op.mult, Op.add)
                u1sq = tw("u1sq")
                nc.gpsimd.tensor_tensor(u1sq[:], u1[:], u1[:], Op.mult)
                D1 = tw("D1")
                nc.vector.tensor_scalar(D1[:], u1sq[:], -2.5, 2.5, Op.mult, Op.add)
                x1 = tw("x1")
                nc.vector.tensor_mul(x1[:], h1p[:], ab[:])
                x2 = tw("x2")
                nc.vector.tensor_mul(x2[:], h1[:], D1[:])
                ta = tw("ta")
                nc.vector.tensor_add(ta[:], x1[:], x2[:])
                x3 = tw("x3")
                nc.vector.tensor_mul(x3[:], h1p[:], be[:])
                tb = tw("tb")
                nc.vector.tensor_sub(tb[:], x3[:], x2[:])
                hbe = tw("hbe")
                nc.vector.tensor_mul(hbe[:], h1[:], be[:])
                y1 = tw("y1")
                nc.vector.tensor_mul(y1[:], tb[:], sv[:])
                y2 = tw("y2")
                nc.vector.tensor_mul(y2[:], hbe[:], invs[:])
                y3 = tw("y3")
                nc.vector.tensor_add(y3[:], y1[:], y2[:])
                y4 = tw("y4")
                nc.vector.tensor_add(y4[:], ta[:], y3[:])
                etp = tw("etp")
                nc.vector.tensor_mul(etp[:], petT[:], y4[:])
                z1 = tw("z1")
                nc.vector.tensor_mul(z1[:], tb[:], Ee[:])
                z3a = tw("z3a")
                nc.vector.tensor_mul(z3a[:], FQ[:], Ee[:])
                z3 = tw("z3")
                nc.vector.tensor_mul(z3[:], hbe[:], z3a[:])
                z4 = tw("z4")
                nc.vector.tensor_add(z4[:], ta[:], z1[:])
                qsp1 = tw("qsp1")
                nc.vector.tensor_mul(qsp1[:], qmaxT[:], z4[:])
                qsp = tw("qsp")
                nc.vector.tensor_add(qsp[:], qsp1[:], z3[:])
                w1t = tw("w1t")
                nc.vector.tensor_mul(w1t[:], ta[:], dd[:])
                qfp = tw("qfp")
                nc.vector.tensor_add(qfp[:], w1t[:], s1a[:])
                j1 = tw("j1")
                nc.vector.tensor_add(j1[:], etp[:], qsp[:])
                j2 = tw("j2")
                nc.vector.tensor_add(j2[:], j1[:], qfp[:])
                j3 = tw("j3")
                nc.vector.tensor_scalar(j3[:], j2[:], -1.0, 1.0, Op.mult, Op.add)
                Jt = tw("s1J")
                nc.vector.tensor_scalar(Jt[:], j3[:], -1.0, 1.02, Op.max, Op.min)
                scp = wpool.tile([PP, 2 * L], F32, tag="scp", name="scp")
                nc.vector.tensor_tensor_scan(
                    scp[:, L : 2 * L], Jt[:], ones[:], 1.0, Op.mult, Op.mult
                )
                nc.vector.tensor_tensor_scan(
                    scp[:, 0:L], Jt[:], rr[:], 0.0, Op.mult, Op.add
                )
                pre_s = tw("pre_s")
                nc.vector.tensor_add(pre_s[:], S1[:], scp[:, 0:L])
                ds = boundary_fix(scp)
                apply_delta(S1, SP1, scp, pre_s, ds)

        # ---- final streamflow from post-update soil state ----
        u0q = wpool.tile([PP, L], F32, tag="u0", name="u0q")
        nc.scalar.activation(u0q[:], S1[:], Act.Tanh, scale=5.0)
        dq = wpool.tile([PP, L], F32, tag="dd", name="dq")
        nc.vector.tensor_sub(dq[:], S1[:], smaxT[:])
        u1q = wpool.tile([PP, L], F32, tag="u1", name="u1q")
        nc.scalar.activation(u1q[:], dq[:], Act.Tanh, scale=5.0)
        argq = wpool.tile([PP, L], F32, tag="ea", name="argq")
        nc.vector.tensor_mul(argq[:], fT[:], dq[:])
        Eq = wpool.tile([PP, L], F32, tag="Ee", name="Eq")
        nc.scalar.activation(Eq[:], argq[:], Act.Exp)
        h1q = wpool.tile([PP, L], F32, tag="h1", name="h1q")
        nc.vector.tensor_scalar(h1q[:], u0q[:], 0.5, 0.5, Op.mult, Op.add)
        abq = wpool.tile([PP, L], F32, tag="ab", name="abq")
        nc.vector.tensor_scalar(abq[:], u1q[:], 0.5, 0.5, Op.mult, Op.add)
        beq = wpool.tile([PP, L], F32, tag="be", name="beq")
        nc.vector.tensor_scalar(beq[:], u1q[:], -0.5, 0.5, Op.mult, Op.add)
        qq1 = wpool.tile([PP, L], F32, tag="q1", name="qq1")
        nc.vector.tensor_mul(qq1[:], beq[:], Eq[:])
        qq2 = wpool.tile([PP, L], F32, tag="q2", name="qq2")
        nc.vector.tensor_add(qq2[:], abq[:], qq1[:])
        qq3 = wpool.tile([PP, L], F32, tag="q3", name="qq3")
        nc.vector.tensor_mul(qq3[:], qmaxT[:], qq2[:])
        qsb = wpool.tile([PP, L], F32, tag="qsub", name="qsb")
        nc.vector.tensor_mul(qsb[:], h1q[:], qq3[:])
        hab = wpool.tile([PP, L], F32, tag="s1a", name="hab")
        nc.vector.tensor_mul(hab[:], h1q[:], abq[:])
        qsf = wpool.tile([PP, L], F32, tag="qsurf", name="qsf")
        nc.vector.tensor_mul(qsf[:], hab[:], dq[:])
        qfin = wpool.tile([PP, L], F32, tag="gg", name="qfin")
        nc.vector.tensor_add(qfin[:], qsb[:], qsf[:])
        nc.sync.dma_start(q_out.rearrange("c (b l) -> (c b) l", l=L), qfin[:])


_CACHED = {}


def _get_module():
    if "nc" in _CACHED:
        return _CACHED["nc"]
    nc = bacc.Bacc(
        "TRN2", target_bir_lowering=False, debug=False, num_devices=NCORES
    )
    att = nc.dram_tensor("att", [BC, 60, T], F32R, kind="ExternalInput").ap()
    met = nc.dram_tensor("met", [3, PP, L], F32, kind="ExternalInput").ap()
    w1k = nc.dram_tensor("w1k", [60, H1], F32R, kind="ExternalInput").ap()
    b1 = nc.dram_tensor("b1", [H1], F32, kind="ExternalInput").ap()
    w2r = nc.dram_tensor("w2r", [H1, H2], F32R, kind="ExternalInput").ap()
    w2s = nc.dram_tensor("w2s", [H1, H2], F32R, kind="ExternalInput").ap()
    b2 = nc.dram_tensor("b2", [H2], F32, kind="ExternalInput").ap()
    w3 = nc.dram_tensor("w3", [H2, 6], F32, kind="ExternalInput").ap()
    b3 = nc.dram_tensor("b3", [6], F32, kind="ExternalInput").ap()
    ksm = nc.dram_tensor("ksm", [3, PP, PP], F32, kind="ExternalInput").ap()
    zcm = nc.dram_tensor("zcm", [PP, 3], F32, kind="ExternalInput").ap()
    q = nc.dram_tensor("q", [BC, T], F32, kind="ExternalOutput").ap()
    with tile.TileContext(nc) as tc:
        _build_kernel(tc, [q], [att, met, w1k, b1, w2r, w2s, b2, w3, b3, ksm, zcm])
    nc.compile()
    _CACHED["nc"] = nc
    return nc


def _shard_inputs(inputs):
    """Per-core input dicts: slice the catchment axis; host-side layout
    transforms only (transpose/reshape, no model compute)."""
    ks, zcv = _host_constants()
    xs = np.ascontiguousarray(np.asarray(inputs["inputs"], np.float32))

    def trunc9(a):
        ai = np.ascontiguousarray(a, np.float32).view(np.uint32)
        return (ai & np.uint32(0xFFFFC000)).view(np.float32)

    w1f = np.asarray(inputs["w1"], np.float32)
    w1r_h = trunc9(w1f)
    w1s_h = (w1f - w1r_h).astype(np.float32)
    w1k_h = np.concatenate([w1r_h, w1s_h, w1r_h, w1s_h], axis=0)  # pairs [ar,ar,as,as]
    w2f = np.asarray(inputs["w2"], np.float32)
    w2r_h = trunc9(w2f)
    w2s_h = (w2f - w2r_h).astype(np.float32)
    common = {
        "w1k": np.ascontiguousarray(w1k_h),
        "b1": np.ascontiguousarray(np.asarray(inputs["b1"], np.float32)),
        "w2r": np.ascontiguousarray(w2r_h),
        "w2s": np.ascontiguousarray(w2s_h),
        "b2": np.ascontiguousarray(np.asarray(inputs["b2"], np.float32)),
        "w3": np.ascontiguousarray(np.asarray(inputs["w3"], np.float32)),
        "b3": np.ascontiguousarray(np.asarray(inputs["b3"], np.float32)),
        "ksm": ks,
        "zcm": zcv,
    }
    in_maps = []
    for k in range(NCORES):
        xk = xs[k * BC : (k + 1) * BC]                      # [16, T, 20]
        attf = xk[:, :, 5:20].transpose(0, 2, 1)            # [16, 15, T]
        attr = trunc9(attf)
        atts = (attf - attr).astype(np.float32)
        att = np.ascontiguousarray(
            np.concatenate([attr, attr, atts, atts], axis=1)
        )
        met = np.ascontiguousarray(
            xk[:, :, 0:3].transpose(2, 0, 1).reshape(3, BC, NB, L).reshape(3, PP, L)
        )
        in_maps.append({"att": att, "met": met, **common})
    return in_maps


def kernel(**inputs):
    nc = _get_module()
    in_maps = _shard_inputs(inputs)
    res = bass_utils.run_bass_kernel_spmd(nc, in_maps, core_ids=list(range(NCORES)))
    q = np.concatenate([res.results[k]["q"] for k in range(NCORES)], axis=0)
    return q[:, :, None].astype(np.float32)


if __name__ == "__main__":
    _get_module()
    print("module built OK")



# revision 11
# speedup vs baseline: 1.4327x; 1.4327x over previous
"""Trainium2 Bass kernel for the differentiable EXP-HYDRO module.

Strategy (8 NeuronCores, data-parallel over the catchment axis):
  - Each core gets 16 catchments x 4096 timesteps.
  - Parameterization MLP on the PE in bf16: L1 uses a 3-way bf16 split of
    (attrs, w1) packed into K=46 (exact to ~2e-7, incl. a ones-row for b1),
    L2 is a single-pass bf16 matmul (param err ~6e-5, validated end-to-end
    at rel err ~3e-3 vs the 2e-2 gate), L3 runs fp32 with w3 stationary and
    tokens on N so the [6, tokens] result DMAs straight into the scan
    staging layout.  tanh runs on ACT; a few L1 tiles use a DVE quintic
    polynomial (bf16, 2x mode) to keep ACT off the critical path.
  - The sequential bucket scan is solved parallel-in-time with the
    tensor_tensor_scan fixed-point iteration (frozen-gate snow, clamped
    Newton soil); 26 snow + 2 soil sweeps reach ~3e-3 rel.  Layout:
    [128 partitions = 16 catchments x 8 time-blocks, 512 steps];
    block-boundary carries stitched via Kogge-Stone PE shift matrices.
    Sweep elementwise work is spread across DVE / GpSimd / ACT.
"""

import numpy as np
from contextlib import ExitStack

import ml_dtypes

import concourse.bass as bass
import concourse.bacc as bacc
import concourse.mybir as mybir
import concourse.tile as tile
from concourse import bass_utils

F32 = mybir.dt.float32
BF16 = mybir.dt.bfloat16
Op = mybir.AluOpType
Act = mybir.ActivationFunctionType

B, T, NF = 128, 4096, 20
NCORES = 8
BC = B // NCORES          # catchments per core = 16
NB = 8                    # time blocks per catchment
L = T // NB               # 512 steps per block
PP = BC * NB              # 128 partitions
N_S0 = 26                 # snow-bucket sweeps
N_S1 = 2                  # soil-bucket sweeps
H1, H2 = 256, 64
KA = 46                   # L1 contraction rows: [a_hi, a_lo, a_hi, ones]
C3 = -0.3293018           # quintic tanh fit over |x| <= 0.74
C5 = 0.10453746


def _host_constants():
    ks = np.zeros((3, PP, PP), np.float32)
    zc = np.zeros((PP, 3), np.float32)
    for ki, k in enumerate((1, 2, 4)):
        for mcol in range(PP):
            if (mcol % NB) >= k:
                ks[ki, mcol - k, mcol] = 1.0
        zc[:, ki] = (np.arange(PP) % NB < k).astype(np.float32)
    return ks, zc


def _build_kernel(tc, outs, ins):
    nc = tc.nc
    (att, met, w1k, w2c, b2d, w3, b3b, ksm, zcm) = ins
    q_out = outs[0]

    with ExitStack() as ctx:
        const = ctx.enter_context(tc.tile_pool(name="const", bufs=1))
        spool = ctx.enter_context(tc.tile_pool(name="scan", bufs=1))
        dpool = ctx.enter_context(tc.tile_pool(name="dram", bufs=1, space="DRAM"))

        # ---- constants ----
        w1k_s = const.tile([KA, H1], BF16)
        nc.sync.dma_start(w1k_s[:], w1k[:])
        w2a = const.tile([128, H2], BF16)
        nc.sync.dma_start(w2a[:], w2c[0:128, :])
        w2b = const.tile([128, H2], BF16)
        nc.sync.dma_start(w2b[:], w2c[128:256, :])
        b2d_s = const.tile([128, 1], F32)
        nc.sync.dma_start(b2d_s[:], b2d[:])
        w3s = const.tile([128, 6], BF16)
        nc.sync.dma_start(w3s[0:64, :], w3[:])
        nc.sync.dma_start(w3s[64:128, :], w3[:])
        b3c = const.tile([PP, 6], F32)
        nc.sync.dma_start(b3c[:], b3b[:])
        ks1 = const.tile([PP, PP], F32)
        nc.sync.dma_start(ks1[:], ksm[0])
        ks2 = const.tile([PP, PP], F32)
        nc.sync.dma_start(ks2[:], ksm[1])
        ks4 = const.tile([PP, PP], F32)
        nc.sync.dma_start(ks4[:], ksm[2])
        zc = const.tile([PP, 3], F32)
        nc.sync.dma_start(zc[:], zcm[:])
        ones = const.tile([PP, L], F32)
        nc.vector.memset(ones[:], 1.0)
        cm75 = const.tile([PP, 1], F32)
        nc.vector.memset(cm75[:], -7.5)

        # ---- DRAM staging: per param-row layout [v, l] ----
        params_d = dpool.tile([PP, 6 * L], F32)

        # ---- MLP phase ----
        POLY = {(1, 1), (1, 2), (1, 3)}  # (half, g) L1 tiles done via DVE poly

        with tc.tile_pool(name="mlp_att", bufs=2) as apool, \
             tc.tile_pool(name="mlp_h1", bufs=1) as hpool, \
             tc.tile_pool(name="mlp_h2", bufs=2) as h2pool, \
             tc.tile_pool(name="mlp_w", bufs=2) as mwork, \
             tc.tile_pool(name="ps1", bufs=1, space="PSUM") as ps1p, \
             tc.tile_pool(name="ps2", bufs=1, space="PSUM") as ps2p, \
             tc.tile_pool(name="ps3", bufs=2, space="PSUM") as ps3p:

            def emit_l1(c, ci, half, g, att_t, h1t):
                ps1 = ps1p.tile([128, 1024], F32, tag="ps1", name=f"ps1_{c}_{half}_{g}")
                for hv in (0, 1):
                    nc.tensor.matmul(
                        ps1[:, hv * 512 : (hv + 1) * 512],
                        w1k_s[:, half * 128 : (half + 1) * 128],
                        att_t[:, g * 1024 + hv * 512 : g * 1024 + (hv + 1) * 512],
                        start=True, stop=True,
                    )
                dst = h1t[:, g * 1024 : (g + 1) * 1024]
                if (half, g) in POLY:
                    xb = mwork.tile([128, 1024], BF16, tag="xb", name="xb")
                    nc.vector.tensor_copy(xb[:], ps1[:])
                    s = mwork.tile([128, 1024], BF16, tag="psq", name="psq")
                    nc.vector.tensor_tensor(s[:], xb[:], xb[:], Op.mult)
                    m = mwork.tile([128, 1024], BF16, tag="pm", name="pm")
                    nc.vector.tensor_scalar(m[:], s[:], C5, C3, Op.mult, Op.add)
                    u = mwork.tile([128, 1024], BF16, tag="pu", name="pu")
                    nc.vector.tensor_tensor(u[:], m[:], s[:], Op.mult)
                    nc.vector.scalar_tensor_tensor(dst, u[:], 1.0, xb[:], Op.add, Op.mult)
                else:
                    nc.scalar.activation(dst, ps1[:], Act.Tanh)

            def emit_l2(h1_prev, g):
                ps2 = ps2p.tile([128, 1024], F32, tag="ps2", name=f"ps2_{g}")
                for hv in (0, 1):
                    gsl = slice(g * 1024 + hv * 512, g * 1024 + (hv + 1) * 512)
                    osl = slice(hv * 512, (hv + 1) * 512)
                    nc.tensor.matmul(ps2[0:64, osl], w2a[:], h1_prev[(0, 0)][:, gsl],
                                     start=True, stop=False)
                    nc.tensor.matmul(ps2[0:64, osl], w2b[:], h1_prev[(0, 1)][:, gsl],
                                     start=False, stop=True)
                    nc.tensor.matmul(ps2[64:128, osl], w2a[:], h1_prev[(1, 0)][:, gsl],
                                     start=True, stop=False)
                    nc.tensor.matmul(ps2[64:128, osl], w2b[:], h1_prev[(1, 1)][:, gsl],
                                     start=False, stop=True)
                h2p = h2pool.tile([128, 1024], BF16, tag="h2p", name=f"h2p_{g}")
                nc.scalar.activation(h2p[:], ps2[:], Act.Tanh, bias=b2d_s[:, 0:1])
                return h2p

            def emit_l3(pair, g, h2p, ps3_pair):
                # tokens-on-M: 8 chunks of 128 tokens per g-tile, per catchment
                for ci in (0, 1):
                    rows = slice(ci * 64, ci * 64 + 64)
                    ps3 = ps3_pair[ci]
                    for ch in range(8):
                        gl = g * 8 + ch
                        nc.tensor.matmul(
                            ps3[:, gl * 6 : gl * 6 + 6],
                            h2p[rows, ch * 128 : (ch + 1) * 128],
                            w3s[rows, :],
                            start=True, stop=True,
                        )

            def emit_l3_finalize(pair, ps3_pair):
                for ci in (0, 1):
                    c = 2 * pair + ci
                    u3 = mwork.tile([128, 192], F32, tag=f"u3_{ci}", name=f"u3_{c}")
                    nc.scalar.activation(u3[:], ps3_pair[ci][:], Act.Copy)
                    for b in range(NB):
                        p = c * NB + b
                        dst = params_d[p : p + 1, :].rearrange(
                            "o (i c4 v) -> (o i) c4 v", i=128, c4=4, v=6
                        )
                        src = u3[:, b * 24 : (b + 1) * 24].rearrange(
                            "p (c4 v) -> p c4 v", c4=4
                        )
                        nc.sync.dma_start(dst, src)

            h1_prev = None
            pend_l3 = []
            ps3_cur = None
            for k in range(BC // 2 + 1):
                l1_units = []
                h1_cur = None
                if k > 0:
                    ps3_cur = {
                        ci: ps3p.tile([128, 192], F32, tag=f"ps3_{ci}",
                                      name=f"ps3p_{k - 1}_{ci}")
                        for ci in (0, 1)
                    }
                if k < BC // 2:
                    att_ts = {}
                    for ci in (0, 1):
                        c = 2 * k + ci
                        att_t = apool.tile([KA, T], BF16, tag=f"att{ci}",
                                           name=f"att_{c}")
                        nc.sync.dma_start(att_t[:], att[c])
                        att_ts[ci] = att_t
                    h1_cur = {}
                    for ci in (0, 1):
                        for half in (0, 1):
                            h1_cur[(ci, half)] = hpool.tile(
                                [128, T], BF16, tag=f"h1_{k % 2}_{ci}_{half}",
                                name=f"h1_{k}_{ci}_{half}",
                            )
                    for g in range(4):
                        for ci in (0, 1):
                            for half in (0, 1):
                                l1_units.append(
                                    (2 * k + ci, ci, half, g, att_ts[ci],
                                     h1_cur[(ci, half)])
                                )
                li = 0
                for slot in range(4):
                    for _ in range(4):
                        if li < len(l1_units):
                            emit_l1(*l1_units[li])
                            li += 1
                    # delayed-by-one-slot L3 so PE is not head-of-line blocked
                    if pend_l3:
                        pr, g, h2p, psp = pend_l3.pop(0)
                        emit_l3(pr, g, h2p, psp)
                        if g == 3:
                            emit_l3_finalize(pr, psp)
                    if k > 0:
                        h2p = emit_l2(h1_prev, slot)
                        pend_l3.append((k - 1, slot, h2p, ps3_cur))
                while li < len(l1_units):
                    emit_l1(*l1_units[li])
                    li += 1
                while k == BC // 2 and pend_l3:
                    pr, g, h2p, psp = pend_l3.pop(0)
                    emit_l3(pr, g, h2p, psp)
                    if g == 3:
                        emit_l3_finalize(pr, psp)
                h1_prev = h1_cur

        wpool = ctx.enter_context(tc.tile_pool(name="work", bufs=1))

        # ---- gather to scan layout [128, 512] + fold sigmoid into tanh ----
        with tc.tile_pool(name="gather", bufs=1) as gpool:
            pall = gpool.tile([PP, 6 * L], F32)
            nc.sync.dma_start(pall[:], params_d[:])
            pview = pall.rearrange("p (i c4 v) -> p c4 i v", i=128, c4=4, v=6)
            U = []
            for v in range(6):
                uv = spool.tile([PP, L], F32, name=f"uparam{v}")
                nc.scalar.activation(
                    uv.rearrange("p (c4 i) -> p c4 i", c4=4),
                    pview[:, :, :, v], Act.Tanh,
                    bias=b3c[:, v : v + 1], scale=0.5,
                )
                U.append(uv)
            petT = spool.tile([PP, L], F32)
            nc.sync.dma_start(petT[:], met[0])
            tmT = spool.tile([PP, L], F32)
            nc.sync.dma_start(tmT[:], met[1])
            prT = spool.tile([PP, L], F32)
            nc.sync.dma_start(prT[:], met[2])

            # ---- coefficient precompute ----
            ph = wpool.tile([PP, L], F32, tag="w0", name="ph")
            nc.vector.tensor_scalar_mul(ph[:], prT[:], 0.5)
            wps = wpool.tile([PP, L], F32, tag="w1", name="wps")
            nc.vector.scalar_tensor_tensor(wps[:], U[0][:], -1.5, tmT[:], Op.mult, Op.subtract)
            ups = wpool.tile([PP, L], F32, tag="w2", name="ups")
            nc.scalar.activation(ups[:], wps[:], Act.Tanh, bias=cm75[:], scale=5.0)
            psnow = spool.tile([PP, L], F32)
            nc.vector.scalar_tensor_tensor(psnow[:], ups[:], 1.0, ph[:], Op.add, Op.mult)
            om = wpool.tile([PP, L], F32, tag="w3", name="om")
            nc.vector.tensor_scalar(om[:], ups[:], -1.0, 1.0, Op.mult, Op.add)
            prain = spool.tile([PP, L], F32)
            nc.vector.tensor_mul(prain[:], om[:], ph[:])
            wA = wpool.tile([PP, L], F32, tag="w4", name="wA")
            nc.vector.scalar_tensor_tensor(wA[:], U[1][:], -1.5, tmT[:], Op.mult, Op.add)
            uA = wpool.tile([PP, L], F32, tag="w5", name="uA")
            nc.scalar.activation(uA[:], wA[:], Act.Tanh, bias=cm75[:], scale=5.0)
            Ah2 = spool.tile([PP, L], F32)
            nc.vector.tensor_scalar(Ah2[:], uA[:], 0.25, 0.25, Op.mult, Op.add)
            xm = wpool.tile([PP, L], F32, tag="w6", name="xm")
            nc.vector.tensor_scalar_add(xm[:], wA[:], -1.5)
            d5 = wpool.tile([PP, L], F32, tag="w7", name="d5")
            nc.gpsimd.tensor_scalar(d5[:], U[2][:], 2.5, 2.5, Op.mult, Op.add)
            mT = spool.tile([PP, L], F32)
            nc.vector.tensor_mul(mT[:], d5[:], xm[:])
            fT = spool.tile([PP, L], F32)
            nc.gpsimd.tensor_scalar(fT[:], U[3][:], 0.05, 0.05, Op.mult, Op.add)
            smaxT = spool.tile([PP, L], F32)
            nc.vector.tensor_scalar(smaxT[:], U[4][:], 700.0, 800.0, Op.mult, Op.add)
            qmaxT = spool.tile([PP, L], F32)
            nc.gpsimd.tensor_scalar(qmaxT[:], U[5][:], 20.0, 30.0, Op.mult, Op.add)
            invs = spool.tile([PP, L], F32)
            nc.vector.reciprocal(invs[:], smaxT[:])
            FQ = spool.tile([PP, L], F32)
            nc.vector.tensor_mul(FQ[:], fT[:], qmaxT[:])
            PQ = spool.tile([PP, L], F32)
            nc.vector.tensor_add(PQ[:], petT[:], qmaxT[:])
            psv = spool.tile([PP, L], F32)
            nc.gpsimd.tensor_tensor(psv[:], petT[:], invs[:], Op.mult)

        # ---- state tiles ----
        S0 = spool.tile([PP, L], F32)
        nc.vector.memset(S0[:], 0.0)
        SP0 = spool.tile([PP, L], F32)
        nc.vector.memset(SP0[:], 0.0)
        S1 = spool.tile([PP, L], F32)
        nc.vector.memset(S1[:], 0.0)
        SP1 = spool.tile([PP, L], F32)
        nc.vector.memset(SP1[:], 0.0)
        RT = spool.tile([PP, L], F32)

        with tc.tile_pool(name="ks_ps", bufs=2, space="PSUM") as kpool:

            def boundary_fix(scp):
                """Exclusive block-carry via Kogge-Stone over partitions."""
                cols = scp.rearrange("p (two l) -> p l two", two=2)[:, L - 1, :]
                p_cur, g_cur = cols[:, 0:1], cols[:, 1:2]
                rhs = cols
                for ki, (k, mat) in enumerate(((1, ks1), (2, ks2), (4, ks4))):
                    psr = kpool.tile([PP, 2], F32, tag="psr", name=f"psr{ki}")
                    nc.tensor.matmul(psr[:], mat[:], rhs, start=True, stop=True)
                    gp_n = wpool.tile([PP, 2], F32, tag=f"gpn{ki}", name=f"gpn{ki}")
                    nc.vector.scalar_tensor_tensor(
                        gp_n[:, 1:2], psr[:, 1:2], zc[:, ki : ki + 1], g_cur,
                        Op.add, Op.mult,
                    )
                    nc.vector.scalar_tensor_tensor(
                        gp_n[:, 0:1], psr[:, 0:1], g_cur, p_cur, Op.mult, Op.add
                    )
                    p_cur, g_cur = gp_n[:, 0:1], gp_n[:, 1:2]
                    rhs = gp_n[:]
                psd = kpool.tile([PP, 1], F32, tag="psd", name="psd")
                nc.tensor.matmul(psd[:], ks1[:], p_cur, start=True, stop=True)
                ds = wpool.tile([PP, 1], F32, tag="ksds", name="ksds")
                nc.vector.tensor_copy(ds[:], psd[:])
                return ds

            def apply_delta(S, SP, scp, ds):
                # S += dp + gp*ds ; SP = S shifted by one step (+ carry col)
                delta = wpool.tile([PP, L], F32, tag="delta", name="delta")
                nc.vector.scalar_tensor_tensor(
                    delta[:], scp[:, L : 2 * L], ds[:], scp[:, 0:L], Op.mult, Op.add
                )
                nc.vector.tensor_add(S[:], S[:], delta[:])
                nc.scalar.activation(SP[:, 1:L], S[:, 0 : L - 1], Act.Copy)
                nc.vector.tensor_add(SP[:, 0:1], SP[:, 0:1], ds[:])

            def tw(nm):
                return wpool.tile([PP, L], F32, tag=nm, name=nm)

            # ---- snow bucket sweeps (frozen-gate propagator) ----
            for it in range(N_S0):
                u = tw("u0")
                nc.scalar.activation(u[:], SP0[:], Act.Tanh, scale=5.0)
                mn = tw("mn")
                nc.vector.tensor_tensor(mn[:], SP0[:], mT[:], Op.min)
                ltf = tw("ltf")
                nc.vector.tensor_tensor(ltf[:], SP0[:], mT[:], Op.is_lt)
                AH = tw("AH")
                nc.vector.scalar_tensor_tensor(AH[:], u[:], 1.0, Ah2[:], Op.add, Op.mult)
                melt = tw("melt")
                nc.gpsimd.tensor_tensor(melt[:], AH[:], mn[:], Op.mult)
                jt = tw("jt")
                nc.gpsimd.tensor_tensor(jt[:], AH[:], ltf[:], Op.mult)
                Jt = tw("s1J")
                nc.scalar.activation(Jt[:], jt[:], Act.Copy, bias=1.0, scale=-1.0)
                rr1 = tw("rr1")
                nc.gpsimd.tensor_tensor(rr1[:], psnow[:], melt[:], Op.subtract)
                t2 = tw("t2")
                nc.gpsimd.tensor_tensor(t2[:], SP0[:], S0[:], Op.subtract)
                rr = tw("s1r")
                nc.vector.tensor_add(rr[:], rr1[:], t2[:])
                scp = wpool.tile([PP, 2 * L], F32, tag="scp", name="scp")
                nc.vector.tensor_tensor_scan(
                    scp[:, L : 2 * L], Jt[:], ones[:], 1.0, Op.mult, Op.mult
                )
                nc.vector.tensor_tensor_scan(
                    scp[:, 0:L], Jt[:], rr[:], 0.0, Op.mult, Op.add
                )
                ds = boundary_fix(scp)
                apply_delta(S0, SP0, scp, ds)

            # ---- melt from converged snow state, rain+melt forcing ----
            u = tw("u0")
            nc.scalar.activation(u[:], SP0[:], Act.Tanh, scale=5.0)
            mn = tw("mn")
            nc.vector.tensor_tensor(mn[:], SP0[:], mT[:], Op.min)
            AH = tw("AH")
            nc.vector.scalar_tensor_tensor(AH[:], u[:], 1.0, Ah2[:], Op.add, Op.mult)
            melt = tw("melt")
            nc.vector.tensor_mul(melt[:], AH[:], mn[:])
            nc.vector.tensor_add(RT[:], prain[:], melt[:])

            # ---- soil bucket sweeps (clamped Newton propagator) ----
            for it in range(N_S1):
                dd = tw("dd")
                nc.vector.tensor_sub(dd[:], SP1[:], smaxT[:])
                u0 = tw("u0")
                nc.scalar.activation(u0[:], SP1[:], Act.Tanh, scale=5.0)
                u1 = tw("u1")
                nc.scalar.activation(u1[:], dd[:], Act.Tanh, scale=5.0)
                ea = tw("ea")
                nc.gpsimd.tensor_tensor(ea[:], fT[:], dd[:], Op.mult)
                eac = tw("eac")
                nc.vector.tensor_scalar_min(eac[:], ea[:], 2.0)
                Ee = tw("Ee")
                nc.scalar.activation(Ee[:], eac[:], Act.Exp)
                h1 = tw("h1")
                nc.scalar.activation(h1[:], u0[:], Act.Copy, bias=0.5, scale=0.5)
                ab = tw("ab")
                nc.scalar.activation(ab[:], u1[:], Act.Copy, bias=0.5, scale=0.5)
                be = tw("be")
                nc.scalar.activation(be[:], u1[:], Act.Copy, bias=0.5, scale=-0.5)
                m1 = tw("m1")
                nc.gpsimd.tensor_tensor(m1[:], psv[:], SP1[:], Op.mult)
                qE = tw("qE")
                nc.vector.tensor_mul(qE[:], qmaxT[:], Ee[:])
                m3 = tw("m3")
                nc.vector.tensor_add(m3[:], m1[:], qE[:])
                m4 = tw("m4")
                nc.vector.tensor_mul(m4[:], be[:], m3[:])
                m5 = tw("m5")
                nc.vector.tensor_mul(m5[:], ab[:], PQ[:])
                inner = tw("inner")
                nc.vector.tensor_add(inner[:], m4[:], m5[:])
                etq = tw("etq")
                nc.vector.tensor_mul(etq[:], h1[:], inner[:])
                s1a = tw("s1a")
                nc.gpsimd.tensor_tensor(s1a[:], h1[:], ab[:], Op.mult)
                qsurf = tw("qsurf")
                nc.vector.tensor_mul(qsurf[:], s1a[:], dd[:])
                g1 = tw("g1")
                nc.vector.scalar_tensor_tensor(g1[:], etq[:], -1.0, RT[:], Op.mult, Op.add)
                gg = tw("gg")
                nc.vector.tensor_sub(gg[:], g1[:], qsurf[:])
                t2b = tw("t2")
                nc.gpsimd.tensor_tensor(t2b[:], SP1[:], S1[:], Op.subtract)
                rr = tw("s1r")
                nc.vector.tensor_add(rr[:], gg[:], t2b[:])
                u0sq = tw("u0sq")
                nc.gpsimd.tensor_tensor(u0sq[:], u0[:], u0[:], Op.mult)
                h1p = tw("h1p")
                nc.scalar.activation(h1p[:], u0sq[:], Act.Copy, bias=2.5, scale=-2.5)
                u1sq = tw("u1sq")
                nc.gpsimd.tensor_tensor(u1sq[:], u1[:], u1[:], Op.mult)
                abp = tw("abp")
                nc.scalar.activation(abp[:], u1sq[:], Act.Copy, bias=2.5, scale=-2.5)
                d1 = tw("d1")
                nc.vector.tensor_sub(d1[:], PQ[:], m3[:])
                d2 = tw("d2")
                nc.vector.tensor_mul(d2[:], abp[:], d1[:])
                fE = tw("fE")
                nc.gpsimd.tensor_tensor(fE[:], FQ[:], Ee[:], Op.mult)
                d3 = tw("d3")
                nc.vector.tensor_add(d3[:], psv[:], fE[:])
                d4 = tw("d4")
                nc.vector.tensor_mul(d4[:], be[:], d3[:])
                dinner = tw("dinner")
                nc.vector.tensor_add(dinner[:], d2[:], d4[:])
                jA = tw("jA")
                nc.vector.tensor_mul(jA[:], h1p[:], inner[:])
                jB = tw("jB")
                nc.vector.tensor_mul(jB[:], h1[:], dinner[:])
                jq = tw("jq")
                nc.vector.tensor_add(jq[:], jA[:], jB[:])
                e1 = tw("e1")
                nc.vector.tensor_mul(e1[:], h1p[:], ab[:])
                e2 = tw("e2")
                nc.vector.tensor_mul(e2[:], h1[:], abp[:])
                e3 = tw("e3")
                nc.vector.tensor_add(e3[:], e1[:], e2[:])
                w1d = tw("w1d")
                nc.vector.tensor_mul(w1d[:], e3[:], dd[:])
                jq2 = tw("jq2")
                nc.vector.tensor_add(jq2[:], w1d[:], s1a[:])
                jtot = tw("jtot")
                nc.vector.tensor_add(jtot[:], jq[:], jq2[:])
                j3 = tw("j3")
                nc.scalar.activation(j3[:], jtot[:], Act.Copy, bias=1.0, scale=-1.0)
                Jt = tw("s1J")
                nc.gpsimd.tensor_scalar(Jt[:], j3[:], -1.0, 1.02, Op.max, Op.min)
                scp = wpool.tile([PP, 2 * L], F32, tag="scp", name="scp")
                nc.vector.tensor_tensor_scan(
                    scp[:, L : 2 * L], Jt[:], ones[:], 1.0, Op.mult, Op.mult
                )
                nc.vector.tensor_tensor_scan(
                    scp[:, 0:L], Jt[:], rr[:], 0.0, Op.mult, Op.add
                )
                ds = boundary_fix(scp)
                apply_delta(S1, SP1, scp, ds)

        # ---- final streamflow from post-update soil state ----
        u0q = wpool.tile([PP, L], F32, tag="u0", name="u0q")
        nc.scalar.activation(u0q[:], S1[:], Act.Tanh, scale=5.0)
        dq = wpool.tile([PP, L], F32, tag="dd", name="dq")
        nc.vector.tensor_sub(dq[:], S1[:], smaxT[:])
        u1q = wpool.tile([PP, L], F32, tag="u1", name="u1q")
        nc.scalar.activation(u1q[:], dq[:], Act.Tanh, scale=5.0)
        argq = wpool.tile([PP, L], F32, tag="ea", name="argq")
        nc.gpsimd.tensor_tensor(argq[:], fT[:], dq[:], Op.mult)
        Eq = wpool.tile([PP, L], F32, tag="Ee", name="Eq")
        nc.scalar.activation(Eq[:], argq[:], Act.Exp)
        h1q = wpool.tile([PP, L], F32, tag="h1", name="h1q")
        nc.scalar.activation(h1q[:], u0q[:], Act.Copy, bias=0.5, scale=0.5)
        abq = wpool.tile([PP, L], F32, tag="ab", name="abq")
        nc.scalar.activation(abq[:], u1q[:], Act.Copy, bias=0.5, scale=0.5)
        beq = wpool.tile([PP, L], F32, tag="be", name="beq")
        nc.scalar.activation(beq[:], u1q[:], Act.Copy, bias=0.5, scale=-0.5)
        qq1 = wpool.tile([PP, L], F32, tag="m4", name="qq1")
        nc.vector.tensor_mul(qq1[:], beq[:], Eq[:])
        qq2 = wpool.tile([PP, L], F32, tag="m5", name="qq2")
        nc.vector.tensor_add(qq2[:], abq[:], qq1[:])
        qq3 = wpool.tile([PP, L], F32, tag="m3", name="qq3")
        nc.vector.tensor_mul(qq3[:], qmaxT[:], qq2[:])
        qsb = wpool.tile([PP, L], F32, tag="qE", name="qsb")
        nc.vector.tensor_mul(qsb[:], h1q[:], qq3[:])
        hab = wpool.tile([PP, L], F32, tag="s1a", name="hab")
        nc.gpsimd.tensor_tensor(hab[:], h1q[:], abq[:], Op.mult)
        qsf = wpool.tile([PP, L], F32, tag="qsurf", name="qsf")
        nc.vector.tensor_mul(qsf[:], hab[:], dq[:])
        qfin = wpool.tile([PP, L], F32, tag="gg", name="qfin")
        nc.vector.tensor_add(qfin[:], qsb[:], qsf[:])
        nc.sync.dma_start(q_out.rearrange("c (b l) -> (c b) l", l=L), qfin[:])


_CACHED = {}


def _get_module():
    if "nc" in _CACHED:
        return _CACHED["nc"]
    nc = bacc.Bacc(
        "TRN2", target_bir_lowering=False, debug=False, num_devices=NCORES
    )
    att = nc.dram_tensor("att", [BC, KA, T], BF16, kind="ExternalInput").ap()
    met = nc.dram_tensor("met", [3, PP, L], F32, kind="ExternalInput").ap()
    w1k = nc.dram_tensor("w1k", [KA, H1], BF16, kind="ExternalInput").ap()
    w2c = nc.dram_tensor("w2c", [H1, H2], BF16, kind="ExternalInput").ap()
    b2d = nc.dram_tensor("b2d", [128, 1], F32, kind="ExternalInput").ap()
    w3 = nc.dram_tensor("w3", [H2, 6], BF16, kind="ExternalInput").ap()
    b3b = nc.dram_tensor("b3b", [PP, 6], F32, kind="ExternalInput").ap()
    ksm = nc.dram_tensor("ksm", [3, PP, PP], F32, kind="ExternalInput").ap()
    zcm = nc.dram_tensor("zcm", [PP, 3], F32, kind="ExternalInput").ap()
    q = nc.dram_tensor("q", [BC, T], F32, kind="ExternalOutput").ap()
    with tile.TileContext(nc) as tc:
        _build_kernel(tc, [q], [att, met, w1k, w2c, b2d, w3, b3b, ksm, zcm])
    nc.compile()
    _CACHED["nc"] = nc
    return nc


def _bf16(a):
    return np.asarray(a, np.float32).astype(ml_dtypes.bfloat16)


def _shard_inputs(inputs):
    """Per-core input dicts: slice the catchment axis; host-side layout
    transforms + bf16 splitting only."""
    ks, zcv = _host_constants()
    xs = np.ascontiguousarray(np.asarray(inputs["inputs"], np.float32))

    w1f = np.asarray(inputs["w1"], np.float32)
    b1f = np.asarray(inputs["b1"], np.float32)
    w1hi = _bf16(w1f)
    w1lo = _bf16(w1f - w1hi.astype(np.float32))
    w1k_h = np.concatenate(
        [w1hi, w1hi, w1lo, _bf16(b1f)[None, :]], axis=0
    )  # [46, 256]
    b2f = np.asarray(inputs["b2"], np.float32)
    b3f = np.asarray(inputs["b3"], np.float32)
    common = {
        "w1k": np.ascontiguousarray(w1k_h),
        "w2c": np.ascontiguousarray(_bf16(np.asarray(inputs["w2"], np.float32))),
        "b2d": np.ascontiguousarray(
            np.concatenate([b2f, b2f]).reshape(128, 1).astype(np.float32)
        ),
        "w3": np.ascontiguousarray(_bf16(np.asarray(inputs["w3"], np.float32))),
        "b3b": np.ascontiguousarray(
            np.broadcast_to(0.5 * b3f, (PP, 6)).astype(np.float32)
        ),
        "ksm": ks,
        "zcm": zcv,
    }
    in_maps = []
    for k in range(NCORES):
        xk = xs[k * BC : (k + 1) * BC]                      # [16, T, 20]
        attf = xk[:, :, 5:20].transpose(0, 2, 1)            # [16, 15, T]
        a_hi = _bf16(attf)
        a_lo = _bf16(attf - a_hi.astype(np.float32))
        ones_row = np.ones((BC, 1, T), ml_dtypes.bfloat16)
        att = np.ascontiguousarray(
            np.concatenate([a_hi, a_lo, a_hi, ones_row], axis=1)
        )  # [16, 46, T] rows pair with w1k rows [w_hi, w_hi, w_lo, b1]
        met = np.ascontiguousarray(
            xk[:, :, 0:3].transpose(2, 0, 1).reshape(3, BC, NB, L).reshape(3, PP, L)
        )
        in_maps.append({"att": att, "met": met, **common})
    return in_maps


def kernel(**inputs):
    nc = _get_module()
    in_maps = _shard_inputs(inputs)
    res = bass_utils.run_bass_kernel_spmd(nc, in_maps, core_ids=list(range(NCORES)))
    q = np.concatenate([res.results[k]["q"] for k in range(NCORES)], axis=0)
    return q[:, :, None].astype(np.float32)


if __name__ == "__main__":
    _get_module()
    print("module built OK")


# revision 12
# speedup vs baseline: 1.7879x; 1.2479x over previous
"""Trainium2 Bass kernel for the differentiable EXP-HYDRO module.

Strategy (8 NeuronCores, data-parallel over the catchment axis):
  - Each core gets 16 catchments x 4096 timesteps.
  - Parameterization MLP on the PE in bf16: L1 uses a 3-way bf16 split of
    (attrs, w1) packed into K=46 (exact to ~2e-7, incl. a ones-row for b1),
    L2 is a single-pass bf16 matmul, L3 is bf16 tokens-on-M.  End-to-end
    param error ~7e-5, validated through the scan at rel err ~2e-3 vs the
    2e-2 gate.  tanh on ACT.
  - The sequential bucket scan is solved parallel-in-time: fixed-point
    sweeps with hardware tensor_tensor_scan.  The propagator J, its
    within-block cumprod, and the block-carry weights are evaluated one
    state behind (validated: no convergence cost), which takes the whole
    gate chain off the per-sweep critical path.  Block-boundary carries
    use a transpose -> 128-long scan -> transpose-back instead of
    Kogge-Stone rounds.  State is kept as S plus a one-column previous-
    state carry (SP views), sweeps spread across DVE / GpSimd / ACT.
  - 20 snow + 3 soil sweeps reach the MLP-error floor (~2e-3 rel).
"""

import numpy as np
from contextlib import ExitStack

import ml_dtypes

import concourse.bass as bass
import concourse.bacc as bacc
import concourse.mybir as mybir
import concourse.tile as tile
from concourse import bass_utils

F32 = mybir.dt.float32
BF16 = mybir.dt.bfloat16
Op = mybir.AluOpType
Act = mybir.ActivationFunctionType

B, T, NF = 128, 4096, 20
NCORES = 8
BC = B // NCORES          # catchments per core = 16
NB = 8                    # time blocks per catchment
L = T // NB               # 512 steps per block
PP = BC * NB              # 128 partitions
N_S0 = 20                 # snow-bucket sweeps
N_S1 = 3                  # soil-bucket sweeps
H1, H2 = 256, 64
KA = 46                   # L1 contraction rows: [a_hi, a_lo, a_hi, ones]


def _host_constants():
    # masked identity: diag 0 at block positions p%NB == NB-1 (no carry
    # across catchment boundaries in the block-carry scan)
    idm = np.diag((np.arange(PP) % NB != NB - 1).astype(np.float32)).astype(np.float32)
    return np.ascontiguousarray(idm)


def _build_kernel(tc, outs, ins):
    nc = tc.nc
    (att, met, w1k, w2c, b2d, w3, b3b, idm) = ins
    q_out = outs[0]

    with ExitStack() as ctx:
        const = ctx.enter_context(tc.tile_pool(name="const", bufs=1))
        spool = ctx.enter_context(tc.tile_pool(name="scan", bufs=1))
        dpool = ctx.enter_context(tc.tile_pool(name="dram", bufs=1, space="DRAM"))

        # ---- constants ----
        w1k_s = const.tile([KA, H1], BF16)
        nc.sync.dma_start(w1k_s[:], w1k[:])
        w2a = const.tile([128, H2], BF16)
        nc.sync.dma_start(w2a[:], w2c[0:128, :])
        w2b = const.tile([128, H2], BF16)
        nc.sync.dma_start(w2b[:], w2c[128:256, :])
        b2d_s = const.tile([128, 1], F32)
        nc.sync.dma_start(b2d_s[:], b2d[:])
        w3s = const.tile([128, 6], BF16)
        nc.sync.dma_start(w3s[0:64, :], w3[:])
        nc.sync.dma_start(w3s[64:128, :], w3[:])
        b3c = const.tile([PP, 6], F32)
        nc.sync.dma_start(b3c[:], b3b[:])
        idm_s = const.tile([PP, PP], F32)
        nc.sync.dma_start(idm_s[:], idm[:])
        onecol = const.tile([1, 1], F32)
        nc.vector.memset(onecol[:], 1.0)
        ones = const.tile([PP, L], F32)
        nc.vector.memset(ones[:], 1.0)
        cm75 = const.tile([PP, 1], F32)
        nc.vector.memset(cm75[:], -7.5)

        # ---- DRAM staging ----
        params_d = dpool.tile([PP, 6 * L], F32)

        # ---- MLP phase ----
        with tc.tile_pool(name="mlp_att", bufs=2) as apool, \
             tc.tile_pool(name="mlp_h1", bufs=1) as hpool, \
             tc.tile_pool(name="mlp_h2", bufs=2) as h2pool, \
             tc.tile_pool(name="mlp_w", bufs=2) as mwork, \
             tc.tile_pool(name="ps1", bufs=1, space="PSUM") as ps1p, \
             tc.tile_pool(name="ps2", bufs=1, space="PSUM") as ps2p, \
             tc.tile_pool(name="ps3", bufs=2, space="PSUM") as ps3p:

            def emit_l1(c, ci, half, g, att_t, h1t):
                ps1 = ps1p.tile([128, 1024], F32, tag="ps1", name=f"ps1_{c}_{half}_{g}")
                for hv in (0, 1):
                    nc.tensor.matmul(
                        ps1[:, hv * 512 : (hv + 1) * 512],
                        w1k_s[:, half * 128 : (half + 1) * 128],
                        att_t[:, g * 1024 + hv * 512 : g * 1024 + (hv + 1) * 512],
                        start=True, stop=True,
                    )
                nc.scalar.activation(h1t[:, g * 1024 : (g + 1) * 1024], ps1[:], Act.Tanh)

            def emit_l2(h1_prev, g):
                ps2 = ps2p.tile([128, 1024], F32, tag="ps2", name=f"ps2_{g}")
                # weight-major order: one LDW for w2a (4 matmuls), one for w2b
                for hv in (0, 1):
                    gsl = slice(g * 1024 + hv * 512, g * 1024 + (hv + 1) * 512)
                    osl = slice(hv * 512, (hv + 1) * 512)
                    nc.tensor.matmul(ps2[0:64, osl], w2a[:], h1_prev[(0, 0)][:, gsl],
                                     start=True, stop=False)
                    nc.tensor.matmul(ps2[64:128, osl], w2a[:], h1_prev[(1, 0)][:, gsl],
                                     start=True, stop=False)
                for hv in (0, 1):
                    gsl = slice(g * 1024 + hv * 512, g * 1024 + (hv + 1) * 512)
                    osl = slice(hv * 512, (hv + 1) * 512)
                    nc.tensor.matmul(ps2[0:64, osl], w2b[:], h1_prev[(0, 1)][:, gsl],
                                     start=False, stop=True)
                    nc.tensor.matmul(ps2[64:128, osl], w2b[:], h1_prev[(1, 1)][:, gsl],
                                     start=False, stop=True)
                h2p = h2pool.tile([128, 1024], BF16, tag="h2p", name=f"h2p_{g}")
                nc.scalar.activation(h2p[:], ps2[:], Act.Tanh, bias=b2d_s[:, 0:1])
                return h2p

            def emit_l3(pair, g, h2p, ps3_pair):
                for ci in (0, 1):
                    rows = slice(ci * 64, ci * 64 + 64)
                    ps3 = ps3_pair[ci]
                    for ch in range(8):
                        gl = g * 8 + ch
                        nc.tensor.matmul(
                            ps3[:, gl * 6 : gl * 6 + 6],
                            h2p[rows, ch * 128 : (ch + 1) * 128],
                            w3s[rows, :],
                            start=True, stop=True,
                        )

            def emit_l3_finalize(pair, ps3_pair):
                for ci in (0, 1):
                    c = 2 * pair + ci
                    u3 = mwork.tile([128, 192], F32, tag=f"u3_{ci}", name=f"u3_{c}")
                    nc.scalar.activation(u3[:], ps3_pair[ci][:], Act.Copy)
                    for b in range(NB):
                        p = c * NB + b
                        dst = params_d[p : p + 1, :].rearrange(
                            "o (i c4 v) -> (o i) c4 v", i=128, c4=4, v=6
                        )
                        src = u3[:, b * 24 : (b + 1) * 24].rearrange(
                            "p (c4 v) -> p c4 v", c4=4
                        )
                        nc.sync.dma_start(dst, src)

            h1_prev = None
            pend_l3 = []
            ps3_cur = None
            for k in range(BC // 2 + 1):
                l1_units = []
                h1_cur = None
                if k > 0:
                    ps3_cur = {
                        ci: ps3p.tile([128, 192], F32, tag=f"ps3_{ci}",
                                      name=f"ps3p_{k - 1}_{ci}")
                        for ci in (0, 1)
                    }
                if k < BC // 2:
                    att_ts = {}
                    for ci in (0, 1):
                        c = 2 * k + ci
                        att_t = apool.tile([KA, T], BF16, tag=f"att{ci}",
                                           name=f"att_{c}")
                        nc.sync.dma_start(att_t[:], att[c])
                        att_ts[ci] = att_t
                    h1_cur = {}
                    for ci in (0, 1):
                        for half in (0, 1):
                            h1_cur[(ci, half)] = hpool.tile(
                                [128, T], BF16, tag=f"h1_{k % 2}_{ci}_{half}",
                                name=f"h1_{k}_{ci}_{half}",
                            )
                    # (ci, half)-major so consecutive L1 matmuls share lhsT
                    for ci in (0, 1):
                        for half in (0, 1):
                            for g in range(4):
                                l1_units.append(
                                    (2 * k + ci, ci, half, g, att_ts[ci],
                                     h1_cur[(ci, half)])
                                )
                li = 0
                for slot in range(4):
                    for _ in range(4):
                        if li < len(l1_units):
                            emit_l1(*l1_units[li])
                            li += 1
                    if pend_l3:
                        pr, g, h2p, psp = pend_l3.pop(0)
                        emit_l3(pr, g, h2p, psp)
                        if g == 3:
                            emit_l3_finalize(pr, psp)
                    if k > 0:
                        h2p = emit_l2(h1_prev, slot)
                        pend_l3.append((k - 1, slot, h2p, ps3_cur))
                while li < len(l1_units):
                    emit_l1(*l1_units[li])
                    li += 1
                while k == BC // 2 and pend_l3:
                    pr, g, h2p, psp = pend_l3.pop(0)
                    emit_l3(pr, g, h2p, psp)
                    if g == 3:
                        emit_l3_finalize(pr, psp)
                h1_prev = h1_cur

        wpool = ctx.enter_context(tc.tile_pool(name="work", bufs=1))

        # ---- gather + sigmoid fold (tanh with b3 bias) ----
        with tc.tile_pool(name="gather", bufs=1) as gpool:
            pall = gpool.tile([PP, 6 * L], F32)
            nc.sync.dma_start(pall[:], params_d[:])
            pview = pall.rearrange("p (i c4 v) -> p c4 i v", i=128, c4=4, v=6)
            U = []
            for v in range(6):
                uv = spool.tile([PP, L], F32, name=f"uparam{v}")
                nc.scalar.activation(
                    uv.rearrange("p (c4 i) -> p c4 i", c4=4),
                    pview[:, :, :, v], Act.Tanh,
                    bias=b3c[:, v : v + 1], scale=0.5,
                )
                U.append(uv)
            petT = spool.tile([PP, L], F32)
            nc.sync.dma_start(petT[:], met[0])
            tmT = spool.tile([PP, L], F32)
            nc.sync.dma_start(tmT[:], met[1])
            prT = spool.tile([PP, L], F32)
            nc.sync.dma_start(prT[:], met[2])

            # ---- coefficient precompute ----
            ph = wpool.tile([PP, L], F32, tag="w0", name="ph")
            nc.vector.tensor_scalar_mul(ph[:], prT[:], 0.5)
            wps = wpool.tile([PP, L], F32, tag="w1", name="wps")
            nc.vector.scalar_tensor_tensor(wps[:], U[0][:], -1.5, tmT[:], Op.mult, Op.subtract)
            ups = wpool.tile([PP, L], F32, tag="w2", name="ups")
            nc.scalar.activation(ups[:], wps[:], Act.Tanh, bias=cm75[:], scale=5.0)
            psnow = spool.tile([PP, L], F32)
            nc.vector.scalar_tensor_tensor(psnow[:], ups[:], 1.0, ph[:], Op.add, Op.mult)
            om = wpool.tile([PP, L], F32, tag="w3", name="om")
            nc.vector.tensor_scalar(om[:], ups[:], -1.0, 1.0, Op.mult, Op.add)
            prain = spool.tile([PP, L], F32)
            nc.vector.tensor_mul(prain[:], om[:], ph[:])
            wA = wpool.tile([PP, L], F32, tag="w4", name="wA")
            nc.vector.scalar_tensor_tensor(wA[:], U[1][:], -1.5, tmT[:], Op.mult, Op.add)
            uA = wpool.tile([PP, L], F32, tag="w5", name="uA")
            nc.scalar.activation(uA[:], wA[:], Act.Tanh, bias=cm75[:], scale=5.0)
            Ah2 = spool.tile([PP, L], F32)
            nc.vector.tensor_scalar(Ah2[:], uA[:], 0.25, 0.25, Op.mult, Op.add)
            xm = wpool.tile([PP, L], F32, tag="w6", name="xm")
            nc.vector.tensor_scalar_add(xm[:], wA[:], -1.5)
            d5 = wpool.tile([PP, L], F32, tag="w7", name="d5")
            nc.vector.tensor_scalar(d5[:], U[2][:], 2.5, 2.5, Op.mult, Op.add)
            mT = wpool.tile([PP, L], F32, tag="w8", name="mTt")
            nc.vector.tensor_mul(mT[:], d5[:], xm[:])
            AM = spool.tile([PP, L], F32)
            nc.vector.tensor_mul(AM[:], Ah2[:], mT[:])
            fT = spool.tile([PP, L], F32)
            nc.vector.tensor_scalar(fT[:], U[3][:], 0.05, 0.05, Op.mult, Op.add)
            smaxT = spool.tile([PP, L], F32)
            nc.vector.tensor_scalar(smaxT[:], U[4][:], 700.0, 800.0, Op.mult, Op.add)
            qmaxT = spool.tile([PP, L], F32)
            nc.vector.tensor_scalar(qmaxT[:], U[5][:], 20.0, 30.0, Op.mult, Op.add)
            invs = spool.tile([PP, L], F32)
            nc.vector.reciprocal(invs[:], smaxT[:])
            FQ = spool.tile([PP, L], F32)
            nc.vector.tensor_mul(FQ[:], fT[:], qmaxT[:])
            PQ = spool.tile([PP, L], F32)
            nc.vector.tensor_add(PQ[:], petT[:], qmaxT[:])
            psv = spool.tile([PP, L], F32)
            nc.gpsimd.tensor_tensor(psv[:], petT[:], invs[:], Op.mult)

        # ---- state tiles ----
        S0 = spool.tile([PP, L], F32)
        nc.vector.memset(S0[:], 0.0)
        SP0c = spool.tile([PP, 1], F32)
        nc.vector.memset(SP0c[:], 0.0)
        S1 = spool.tile([PP, L], F32)
        nc.vector.memset(S1[:], 0.0)
        SP1c = spool.tile([PP, 1], F32)
        nc.vector.memset(SP1c[:], 0.0)
        RT = spool.tile([PP, L], F32)
        dsr = spool.tile([1, PP], F32)
        nc.vector.memset(dsr[:], 0.0)

        def tw(nm):
            return wpool.tile([PP, L], F32, tag=nm, name=nm)

        with tc.tile_pool(name="ks_ps", bufs=2, space="PSUM") as kpool:

            def gate_scan(Jn, jtag):
                """cumprod of J, block-end row (masked) for the carry scan."""
                gpn = wpool.tile([PP, L], F32, tag=f"gp{jtag}", name=f"gp{jtag}")
                nc.vector.tensor_tensor_scan(
                    gpn[:], Jn[:], ones[:], 1.0, Op.mult, Op.mult
                )
                gT = kpool.tile([1, PP], F32, tag="gT", name=f"gT{jtag}")
                nc.tensor.matmul(gT[:], gpn[:, L - 1 : L], idm_s[:], start=True, stop=True)
                gmn = wpool.tile([1, PP], F32, tag=f"gm{jtag}", name=f"gm{jtag}")
                nc.vector.tensor_copy(gmn[:], gT[:])
                return gpn, gmn

            def ks_tail(dp, gmask):
                """exclusive block-carry: transpose ends, 128-long scan, back."""
                dT = kpool.tile([1, PP], F32, tag="dT", name="dT")
                nc.tensor.matmul(dT[:], dp[:, L - 1 : L], idm_s[:], start=True, stop=True)
                nc.vector.tensor_tensor_scan(
                    dsr[:, 1:PP], gmask[:, 0 : PP - 1], dT[:, 0 : PP - 1],
                    0.0, Op.mult, Op.add,
                )
                dcol = kpool.tile([PP, 1], F32, tag="dcol", name="dcol")
                nc.tensor.matmul(dcol[:], dsr[:], onecol[:], start=True, stop=True)
                dsb = wpool.tile([PP, 1], F32, tag="ksds", name="ksds")
                nc.vector.tensor_copy(dsb[:], dcol[:])
                return dsb

            # ---- snow: J/gate init at zero state ----
            sgA = tw("sg")
            nc.scalar.activation(sgA[:], AM[:], Act.Sign)
            lt3 = tw("lt3")
            nc.vector.scalar_tensor_tensor(lt3[:], sgA[:], 1.0, Ah2[:], Op.add, Op.mult)
            Jt = wpool.tile([PP, L], F32, tag="s0J", name="J0")
            nc.vector.tensor_scalar(Jt[:], lt3[:], -0.5, 1.0, Op.mult, Op.add)
            gp, gmask = gate_scan(Jt, "s0")

            # ---- snow sweeps (frozen-gate propagator, one-state-stale J) ----
            for it in range(N_S0):
                last = it == N_S0 - 1
                u = tw("u0")
                nc.scalar.activation(u[:, 1:L], S0[:, 0 : L - 1], Act.Tanh, scale=5.0)
                nc.scalar.activation(u[:, 0:1], SP0c[:], Act.Tanh, scale=5.0)
                z = tw("z")
                nc.gpsimd.tensor_tensor(z[:, 1:L], Ah2[:, 1:L], S0[:, 0 : L - 1], Op.mult)
                nc.gpsimd.tensor_tensor(z[:, 0:1], Ah2[:, 0:1], SP0c[:], Op.mult)
                t2 = tw("t2")
                nc.gpsimd.tensor_tensor(t2[:, 1:L], S0[:, 0 : L - 1], S0[:, 1:L], Op.subtract)
                nc.gpsimd.tensor_tensor(t2[:, 0:1], SP0c[:], S0[:, 0:1], Op.subtract)
                rr2 = tw("rr2")
                nc.gpsimd.tensor_tensor(rr2[:], psnow[:], t2[:], Op.add)
                mn2 = tw("mn")
                nc.vector.tensor_tensor(mn2[:], z[:], AM[:], Op.min)
                melt = tw("melt")
                nc.vector.scalar_tensor_tensor(melt[:], u[:], 1.0, mn2[:], Op.add, Op.mult)
                rr = tw("s1r")
                nc.vector.tensor_sub(rr[:], rr2[:], melt[:])
                dp = wpool.tile([PP, L], F32, tag="dp", name="dp")
                nc.vector.tensor_tensor_scan(dp[:], Jt[:], rr[:], 0.0, Op.mult, Op.add)
                dsb = ks_tail(dp, gmask)
                delta = tw("delta")
                nc.vector.scalar_tensor_tensor(delta[:], gp[:], dsb[:], dp[:], Op.mult, Op.add)
                nc.vector.tensor_add(S0[:], S0[:], delta[:])
                nc.vector.tensor_add(SP0c[:], SP0c[:], dsb[:])
                if not last:
                    # gate refresh from this sweep's entry state (stale by one)
                    dz = tw("dz")
                    nc.gpsimd.tensor_tensor(dz[:], mn2[:], z[:], Op.subtract)
                    sg = tw("sg")
                    nc.scalar.activation(sg[:], dz[:], Act.Sign)
                    lt3 = tw("lt3")
                    nc.vector.scalar_tensor_tensor(lt3[:], sg[:], 1.0, Ah2[:], Op.add, Op.mult)
                    jtx = tw("jtx")
                    nc.vector.scalar_tensor_tensor(jtx[:], u[:], 1.0, lt3[:], Op.add, Op.mult)
                    Jt = wpool.tile([PP, L], F32, tag="s0J", name=f"J{it + 1}")
                    nc.vector.tensor_scalar(Jt[:], jtx[:], -0.5, 1.0, Op.mult, Op.add)
                    gp, gmask = gate_scan(Jt, "s0")

            # ---- melt from converged snow state, rain+melt forcing ----
            u = tw("u0")
            nc.scalar.activation(u[:, 1:L], S0[:, 0 : L - 1], Act.Tanh, scale=5.0)
            nc.scalar.activation(u[:, 0:1], SP0c[:], Act.Tanh, scale=5.0)
            z = tw("z")
            nc.gpsimd.tensor_tensor(z[:, 1:L], Ah2[:, 1:L], S0[:, 0 : L - 1], Op.mult)
            nc.gpsimd.tensor_tensor(z[:, 0:1], Ah2[:, 0:1], SP0c[:], Op.mult)
            mn2 = tw("mn")
            nc.vector.tensor_tensor(mn2[:], z[:], AM[:], Op.min)
            melt = tw("melt")
            nc.vector.scalar_tensor_tensor(melt[:], u[:], 1.0, mn2[:], Op.add, Op.mult)
            nc.vector.tensor_add(RT[:], prain[:], melt[:])

            # ---- soil: J init at zero state:  J = 1 - 0.5*pet/smax ----
            Jt1 = wpool.tile([PP, L], F32, tag="s1J", name="J1_0")
            nc.vector.tensor_scalar(Jt1[:], psv[:], -0.5, 1.0, Op.mult, Op.add)
            gp1, gmask1 = gate_scan(Jt1, "s1")

            # ---- soil sweeps (clamped Newton, one-state-stale J) ----
            for it in range(N_S1):
                last = it == N_S1 - 1
                u0 = tw("u0")
                nc.scalar.activation(u0[:, 1:L], S1[:, 0 : L - 1], Act.Tanh, scale=5.0)
                nc.scalar.activation(u0[:, 0:1], SP1c[:], Act.Tanh, scale=5.0)
                dd = tw("dd")
                nc.gpsimd.tensor_tensor(dd[:, 1:L], S1[:, 0 : L - 1], smaxT[:, 1:L], Op.subtract)
                nc.gpsimd.tensor_tensor(dd[:, 0:1], SP1c[:], smaxT[:, 0:1], Op.subtract)
                m1 = tw("m1")
                nc.gpsimd.tensor_tensor(m1[:, 1:L], psv[:, 1:L], S1[:, 0 : L - 1], Op.mult)
                nc.gpsimd.tensor_tensor(m1[:, 0:1], psv[:, 0:1], SP1c[:], Op.mult)
                t2 = tw("t2")
                nc.gpsimd.tensor_tensor(t2[:, 1:L], S1[:, 0 : L - 1], S1[:, 1:L], Op.subtract)
                nc.gpsimd.tensor_tensor(t2[:, 0:1], SP1c[:], S1[:, 0:1], Op.subtract)
                u1 = tw("u1")
                nc.scalar.activation(u1[:], dd[:], Act.Tanh, scale=5.0)
                ea = tw("ea")
                nc.gpsimd.tensor_tensor(ea[:], fT[:], dd[:], Op.mult)
                eac = tw("eac")
                nc.vector.tensor_scalar_min(eac[:], ea[:], 2.0)
                Ee = tw("Ee")
                nc.scalar.activation(Ee[:], eac[:], Act.Exp)
                h1 = tw("h1")
                nc.scalar.activation(h1[:], u0[:], Act.Copy, bias=0.5, scale=0.5)
                ab = tw("ab")
                nc.scalar.activation(ab[:], u1[:], Act.Copy, bias=0.5, scale=0.5)
                be = tw("be")
                nc.scalar.activation(be[:], u1[:], Act.Copy, bias=0.5, scale=-0.5)
                qE = tw("qE")
                nc.vector.tensor_mul(qE[:], qmaxT[:], Ee[:])
                m3 = tw("m3")
                nc.vector.tensor_add(m3[:], m1[:], qE[:])
                m4 = tw("m4")
                nc.vector.tensor_mul(m4[:], be[:], m3[:])
                m5 = tw("m5")
                nc.vector.tensor_mul(m5[:], ab[:], PQ[:])
                inner = tw("inner")
                nc.vector.tensor_add(inner[:], m4[:], m5[:])
                etq = tw("etq")
                nc.vector.tensor_mul(etq[:], h1[:], inner[:])
                s1a = tw("s1a")
                nc.gpsimd.tensor_tensor(s1a[:], h1[:], ab[:], Op.mult)
                qsurf = tw("qsurf")
                nc.vector.tensor_mul(qsurf[:], s1a[:], dd[:])
                g1 = tw("g1")
                nc.vector.scalar_tensor_tensor(g1[:], etq[:], -1.0, RT[:], Op.mult, Op.add)
                gg = tw("gg")
                nc.vector.tensor_sub(gg[:], g1[:], qsurf[:])
                rr = tw("s1r")
                nc.vector.tensor_add(rr[:], gg[:], t2[:])
                dp = wpool.tile([PP, L], F32, tag="dp", name="dp1")
                nc.vector.tensor_tensor_scan(dp[:], Jt1[:], rr[:], 0.0, Op.mult, Op.add)
                dsb = ks_tail(dp, gmask1)
                delta = tw("delta")
                nc.vector.scalar_tensor_tensor(delta[:], gp1[:], dsb[:], dp[:], Op.mult, Op.add)
                nc.vector.tensor_add(S1[:], S1[:], delta[:])
                nc.vector.tensor_add(SP1c[:], SP1c[:], dsb[:])
                if not last:
                    u0sq = tw("u0sq")
                    nc.gpsimd.tensor_tensor(u0sq[:], u0[:], u0[:], Op.mult)
                    h1p = tw("h1p")
                    nc.scalar.activation(h1p[:], u0sq[:], Act.Copy, bias=2.5, scale=-2.5)
                    u1sq = tw("u1sq")
                    nc.gpsimd.tensor_tensor(u1sq[:], u1[:], u1[:], Op.mult)
                    abp = tw("abp")
                    nc.scalar.activation(abp[:], u1sq[:], Act.Copy, bias=2.5, scale=-2.5)
                    d1 = tw("d1")
                    nc.vector.tensor_sub(d1[:], PQ[:], m3[:])
                    d2 = tw("d2")
                    nc.vector.tensor_mul(d2[:], abp[:], d1[:])
                    fE = tw("fE")
                    nc.gpsimd.tensor_tensor(fE[:], FQ[:], Ee[:], Op.mult)
                    d3 = tw("d3")
                    nc.vector.tensor_add(d3[:], psv[:], fE[:])
                    d4 = tw("d4")
                    nc.vector.tensor_mul(d4[:], be[:], d3[:])
                    dinner = tw("dinner")
                    nc.vector.tensor_add(dinner[:], d2[:], d4[:])
                    jA = tw("jA")
                    nc.vector.tensor_mul(jA[:], h1p[:], inner[:])
                    jB = tw("jB")
                    nc.vector.tensor_mul(jB[:], h1[:], dinner[:])
                    jq = tw("jq")
                    nc.vector.tensor_add(jq[:], jA[:], jB[:])
                    e1 = tw("e1")
                    nc.vector.tensor_mul(e1[:], h1p[:], ab[:])
                    e2 = tw("e2")
                    nc.vector.tensor_mul(e2[:], h1[:], abp[:])
                    e3 = tw("e3")
                    nc.vector.tensor_add(e3[:], e1[:], e2[:])
                    w1d = tw("w1d")
                    nc.vector.tensor_mul(w1d[:], e3[:], dd[:])
                    jq2 = tw("jq2")
                    nc.vector.tensor_add(jq2[:], w1d[:], s1a[:])
                    jtot = tw("jtot")
                    nc.vector.tensor_add(jtot[:], jq[:], jq2[:])
                    j3 = tw("j3")
                    nc.scalar.activation(j3[:], jtot[:], Act.Copy, bias=1.0, scale=-1.0)
                    Jt1 = wpool.tile([PP, L], F32, tag="s1J", name=f"J1_{it + 1}")
                    nc.vector.tensor_scalar(Jt1[:], j3[:], -1.0, 1.02, Op.max, Op.min)
                    gp1, gmask1 = gate_scan(Jt1, "s1")

        # ---- final streamflow from post-update soil state ----
        u0q = wpool.tile([PP, L], F32, tag="u0", name="u0q")
        nc.scalar.activation(u0q[:], S1[:], Act.Tanh, scale=5.0)
        dq = wpool.tile([PP, L], F32, tag="dd", name="dq")
        nc.vector.tensor_sub(dq[:], S1[:], smaxT[:])
        u1q = wpool.tile([PP, L], F32, tag="u1", name="u1q")
        nc.scalar.activation(u1q[:], dq[:], Act.Tanh, scale=5.0)
        argq = wpool.tile([PP, L], F32, tag="ea", name="argq")
        nc.gpsimd.tensor_tensor(argq[:], fT[:], dq[:], Op.mult)
        Eq = wpool.tile([PP, L], F32, tag="Ee", name="Eq")
        nc.scalar.activation(Eq[:], argq[:], Act.Exp)
        h1q = wpool.tile([PP, L], F32, tag="h1", name="h1q")
        nc.scalar.activation(h1q[:], u0q[:], Act.Copy, bias=0.5, scale=0.5)
        abq = wpool.tile([PP, L], F32, tag="ab", name="abq")
        nc.scalar.activation(abq[:], u1q[:], Act.Copy, bias=0.5, scale=0.5)
        beq = wpool.tile([PP, L], F32, tag="be", name="beq")
        nc.scalar.activation(beq[:], u1q[:], Act.Copy, bias=0.5, scale=-0.5)
        qq1 = wpool.tile([PP, L], F32, tag="m4", name="qq1")
        nc.vector.tensor_mul(qq1[:], beq[:], Eq[:])
        qq2 = wpool.tile([PP, L], F32, tag="m5", name="qq2")
        nc.vector.tensor_add(qq2[:], abq[:], qq1[:])
        qq3 = wpool.tile([PP, L], F32, tag="m3", name="qq3")
        nc.vector.tensor_mul(qq3[:], qmaxT[:], qq2[:])
        qsb = wpool.tile([PP, L], F32, tag="qE", name="qsb")
        nc.vector.tensor_mul(qsb[:], h1q[:], qq3[:])
        hab = wpool.tile([PP, L], F32, tag="s1a", name="hab")
        nc.gpsimd.tensor_tensor(hab[:], h1q[:], abq[:], Op.mult)
        qsf = wpool.tile([PP, L], F32, tag="qsurf", name="qsf")
        nc.vector.tensor_mul(qsf[:], hab[:], dq[:])
        qfin = wpool.tile([PP, L], F32, tag="gg", name="qfin")
        nc.vector.tensor_add(qfin[:], qsb[:], qsf[:])
        nc.sync.dma_start(q_out.rearrange("c (b l) -> (c b) l", l=L), qfin[:])


_CACHED = {}


def _get_module():
    if "nc" in _CACHED:
        return _CACHED["nc"]
    nc = bacc.Bacc(
        "TRN2", target_bir_lowering=False, debug=False, num_devices=NCORES
    )
    att = nc.dram_tensor("att", [BC, KA, T], BF16, kind="ExternalInput").ap()
    met = nc.dram_tensor("met", [3, PP, L], F32, kind="ExternalInput").ap()
    w1k = nc.dram_tensor("w1k", [KA, H1], BF16, kind="ExternalInput").ap()
    w2c = nc.dram_tensor("w2c", [H1, H2], BF16, kind="ExternalInput").ap()
    b2d = nc.dram_tensor("b2d", [128, 1], F32, kind="ExternalInput").ap()
    w3 = nc.dram_tensor("w3", [H2, 6], BF16, kind="ExternalInput").ap()
    b3b = nc.dram_tensor("b3b", [PP, 6], F32, kind="ExternalInput").ap()
    idm = nc.dram_tensor("idm", [PP, PP], F32, kind="ExternalInput").ap()
    q = nc.dram_tensor("q", [BC, T], F32, kind="ExternalOutput").ap()
    with tile.TileContext(nc) as tc:
        _build_kernel(tc, [q], [att, met, w1k, w2c, b2d, w3, b3b, idm])
    nc.compile()
    _CACHED["nc"] = nc
    return nc


def _bf16(a):
    return np.asarray(a, np.float32).astype(ml_dtypes.bfloat16)


def _shard_inputs(inputs):
    """Per-core input dicts: slice the catchment axis; host-side layout
    transforms + bf16 splitting only."""
    idm = _host_constants()
    xs = np.ascontiguousarray(np.asarray(inputs["inputs"], np.float32))

    w1f = np.asarray(inputs["w1"], np.float32)
    b1f = np.asarray(inputs["b1"], np.float32)
    w1hi = _bf16(w1f)
    w1lo = _bf16(w1f - w1hi.astype(np.float32))
    w1k_h = np.concatenate(
        [w1hi, w1hi, w1lo, _bf16(b1f)[None, :]], axis=0
    )  # [46, 256]
    b2f = np.asarray(inputs["b2"], np.float32)
    b3f = np.asarray(inputs["b3"], np.float32)
    common = {
        "w1k": np.ascontiguousarray(w1k_h),
        "w2c": np.ascontiguousarray(_bf16(np.asarray(inputs["w2"], np.float32))),
        "b2d": np.ascontiguousarray(
            np.concatenate([b2f, b2f]).reshape(128, 1).astype(np.float32)
        ),
        "w3": np.ascontiguousarray(_bf16(np.asarray(inputs["w3"], np.float32))),
        "b3b": np.ascontiguousarray(
            np.broadcast_to(0.5 * b3f, (PP, 6)).astype(np.float32)
        ),
        "idm": idm,
    }
    in_maps = []
    for k in range(NCORES):
        xk = xs[k * BC : (k + 1) * BC]                      # [16, T, 20]
        attf = xk[:, :, 5:20].transpose(0, 2, 1)            # [16, 15, T]
        a_hi = _bf16(attf)
        a_lo = _bf16(attf - a_hi.astype(np.float32))
        ones_row = np.ones((BC, 1, T), ml_dtypes.bfloat16)
        att = np.ascontiguousarray(
            np.concatenate([a_hi, a_lo, a_hi, ones_row], axis=1)
        )  # [16, 46, T] rows pair with w1k rows [w_hi, w_hi, w_lo, b1]
        met = np.ascontiguousarray(
            xk[:, :, 0:3].transpose(2, 0, 1).reshape(3, BC, NB, L).reshape(3, PP, L)
        )
        in_maps.append({"att": att, "met": met, **common})
    return in_maps


def kernel(**inputs):
    nc = _get_module()
    in_maps = _shard_inputs(inputs)
    res = bass_utils.run_bass_kernel_spmd(nc, in_maps, core_ids=list(range(NCORES)))
    q = np.concatenate([res.results[k]["q"] for k in range(NCORES)], axis=0)
    return q[:, :, None].astype(np.float32)


if __name__ == "__main__":
    _get_module()
    print("module built OK")


# revision 14
# speedup vs baseline: 2.0347x; 1.1380x over previous
"""Trainium2 Bass kernel for the differentiable EXP-HYDRO module.

Strategy (8 NeuronCores, data-parallel over the catchment axis):
  - Each core gets 16 catchments x 4096 timesteps.
  - Parameterization MLP on the PE in bf16: L1 uses a 3-way bf16 split of
    (attrs, w1) packed into K=46 (exact to ~2e-7, incl. a ones-row for b1),
    L2 is a single-pass bf16 matmul, L3 is bf16 tokens-on-M.  End-to-end
    param error ~7e-5, validated through the scan at rel err ~2e-3 vs the
    2e-2 gate.  tanh on ACT.
  - The sequential bucket scan is solved parallel-in-time: fixed-point
    sweeps with hardware tensor_tensor_scan.  The propagator J, its
    within-block cumprod, and the block-carry weights are evaluated one
    state behind (validated: no convergence cost), which takes the whole
    gate chain off the per-sweep critical path.  Block-boundary carries
    use a transpose -> 128-long scan -> transpose-back instead of
    Kogge-Stone rounds.  State is kept as S plus a one-column previous-
    state carry (SP views), sweeps spread across DVE / GpSimd / ACT.
  - 20 snow + 3 soil sweeps reach the MLP-error floor (~2e-3 rel).
"""

import numpy as np
from contextlib import ExitStack

import ml_dtypes

import concourse.bass as bass
import concourse.bacc as bacc
import concourse.mybir as mybir
import concourse.tile as tile
from concourse import bass_utils

F32 = mybir.dt.float32
BF16 = mybir.dt.bfloat16
Op = mybir.AluOpType
Act = mybir.ActivationFunctionType

B, T, NF = 128, 4096, 20
NCORES = 8
BC = B // NCORES          # catchments per core = 16
NB = 8                    # time blocks per catchment
L = T // NB               # 512 steps per block
PP = BC * NB              # 128 partitions
N_S0 = 20                 # snow-bucket sweeps
N_S1 = 3                  # soil-bucket sweeps
H1, H2 = 256, 64
KA = 46                   # L1 contraction rows: [a_hi, a_lo, a_hi, ones]


def _host_constants():
    # masked identity: diag 0 at block positions p%NB == NB-1 (no carry
    # across catchment boundaries in the block-carry scan)
    idm = np.diag((np.arange(PP) % NB != NB - 1).astype(np.float32)).astype(np.float32)
    return np.ascontiguousarray(idm)


def _build_kernel(tc, outs, ins):
    nc = tc.nc
    (att, met, w1k, w2c, b2d, w3, b3b, idm) = ins
    q_out = outs[0]

    with ExitStack() as ctx:
        const = ctx.enter_context(tc.tile_pool(name="const", bufs=1))
        spool = ctx.enter_context(tc.tile_pool(name="scan", bufs=1))
        dpool = ctx.enter_context(tc.tile_pool(name="dram", bufs=1, space="DRAM"))

        # ---- constants ----
        w1k_s = const.tile([KA, H1], BF16)
        nc.sync.dma_start(w1k_s[:], w1k[:])
        w2a = const.tile([128, H2], BF16)
        nc.sync.dma_start(w2a[:], w2c[0:128, :])
        w2b = const.tile([128, H2], BF16)
        nc.sync.dma_start(w2b[:], w2c[128:256, :])
        b2d_s = const.tile([128, 1], F32)
        nc.sync.dma_start(b2d_s[:], b2d[:])
        w3s = const.tile([128, 6], BF16)
        nc.sync.dma_start(w3s[0:64, :], w3[:])
        nc.sync.dma_start(w3s[64:128, :], w3[:])
        b3c = const.tile([PP, 6], F32)
        nc.sync.dma_start(b3c[:], b3b[:])
        idm_s = const.tile([PP, PP], F32)
        nc.sync.dma_start(idm_s[:], idm[:])
        onecol = const.tile([1, 1], F32)
        nc.vector.memset(onecol[:], 1.0)
        ones = const.tile([PP, L], F32)
        nc.vector.memset(ones[:], 1.0)
        cm75 = const.tile([PP, 1], F32)
        nc.vector.memset(cm75[:], -7.5)

        # ---- DRAM staging ----
        params_d = dpool.tile([PP, 6 * L], F32)

        # ---- MLP phase ----
        with tc.tile_pool(name="mlp_att", bufs=2) as apool, \
             tc.tile_pool(name="mlp_h1", bufs=1) as hpool, \
             tc.tile_pool(name="mlp_h2", bufs=2) as h2pool, \
             tc.tile_pool(name="mlp_w", bufs=2) as mwork, \
             tc.tile_pool(name="ps1", bufs=2, space="PSUM") as ps1p, \
             tc.tile_pool(name="ps2", bufs=1, space="PSUM") as ps2p, \
             tc.tile_pool(name="ps3", bufs=2, space="PSUM") as ps3p:

            def emit_l1(c, ci, half, g, att_t, h1t):
                ps1 = ps1p.tile([128, 1024], F32, tag="ps1", name=f"ps1_{c}_{half}_{g}")
                for hv in (0, 1):
                    nc.tensor.matmul(
                        ps1[:, hv * 512 : (hv + 1) * 512],
                        w1k_s[:, half * 128 : (half + 1) * 128],
                        att_t[:, g * 1024 + hv * 512 : g * 1024 + (hv + 1) * 512],
                        start=True, stop=True,
                    )
                nc.scalar.activation(h1t[:, g * 1024 : (g + 1) * 1024], ps1[:], Act.Tanh)

            def emit_l2(h1_prev, g):
                ps2 = ps2p.tile([128, 1024], F32, tag="ps2", name=f"ps2_{g}")
                # weight-major order: one LDW for w2a (4 matmuls), one for w2b
                for hv in (0, 1):
                    gsl = slice(g * 1024 + hv * 512, g * 1024 + (hv + 1) * 512)
                    osl = slice(hv * 512, (hv + 1) * 512)
                    nc.tensor.matmul(ps2[0:64, osl], w2a[:], h1_prev[(0, 0)][:, gsl],
                                     start=True, stop=False)
                    nc.tensor.matmul(ps2[64:128, osl], w2a[:], h1_prev[(1, 0)][:, gsl],
                                     start=True, stop=False)
                for hv in (0, 1):
                    gsl = slice(g * 1024 + hv * 512, g * 1024 + (hv + 1) * 512)
                    osl = slice(hv * 512, (hv + 1) * 512)
                    nc.tensor.matmul(ps2[0:64, osl], w2b[:], h1_prev[(0, 1)][:, gsl],
                                     start=False, stop=True)
                    nc.tensor.matmul(ps2[64:128, osl], w2b[:], h1_prev[(1, 1)][:, gsl],
                                     start=False, stop=True)
                h2p = h2pool.tile([128, 1024], BF16, tag="h2p", name=f"h2p_{g}")
                nc.scalar.activation(h2p[:], ps2[:], Act.Tanh, bias=b2d_s[:, 0:1])
                return h2p

            def emit_l3(pair, g, h2p, ps3_pair):
                for ci in (0, 1):
                    rows = slice(ci * 64, ci * 64 + 64)
                    for ch in range(8):
                        gl = g * 8 + ch
                        nc.tensor.matmul(
                            ps3_pair[:, ci * 192 + gl * 6 : ci * 192 + gl * 6 + 6],
                            h2p[rows, ch * 128 : (ch + 1) * 128],
                            w3s[rows, :],
                            start=True, stop=True,
                        )

            def emit_l3_finalize(pair, ps3_pair):
                for ci in (0, 1):
                    c = 2 * pair + ci
                    u3 = mwork.tile([128, 192], F32, tag=f"u3_{ci}", name=f"u3_{c}")
                    nc.scalar.activation(u3[:], ps3_pair[:, ci * 192 : (ci + 1) * 192], Act.Copy)
                    for b in range(NB):
                        p = c * NB + b
                        dst = params_d[p : p + 1, :].rearrange(
                            "o (i c4 v) -> (o i) c4 v", i=128, c4=4, v=6
                        )
                        src = u3[:, b * 24 : (b + 1) * 24].rearrange(
                            "p (c4 v) -> p c4 v", c4=4
                        )
                        nc.sync.dma_start(dst, src)

            h1_prev = None
            pend_l3 = []
            ps3_cur = None
            for k in range(BC // 2 + 1):
                l1_units = []
                h1_cur = None
                if k > 0:
                    ps3_cur = ps3p.tile([128, 384], F32, tag="ps3",
                                        name=f"ps3p_{k - 1}")
                if k < BC // 2:
                    att_ts = {}
                    for ci in (0, 1):
                        c = 2 * k + ci
                        att_t = apool.tile([KA, T], BF16, tag=f"att{ci}",
                                           name=f"att_{c}")
                        nc.sync.dma_start(att_t[:], att[c])
                        att_ts[ci] = att_t
                    h1_cur = {}
                    for ci in (0, 1):
                        for half in (0, 1):
                            h1_cur[(ci, half)] = hpool.tile(
                                [128, T], BF16, tag=f"h1_{k % 2}_{ci}_{half}",
                                name=f"h1_{k}_{ci}_{half}",
                            )
                    # (ci, half)-major so consecutive L1 matmuls share lhsT
                    for ci in (0, 1):
                        for half in (0, 1):
                            for g in range(4):
                                l1_units.append(
                                    (2 * k + ci, ci, half, g, att_ts[ci],
                                     h1_cur[(ci, half)])
                                )
                li = 0
                for slot in range(4):
                    for _ in range(4):
                        if li < len(l1_units):
                            emit_l1(*l1_units[li])
                            li += 1
                    if pend_l3:
                        pr, g, h2p, psp = pend_l3.pop(0)
                        emit_l3(pr, g, h2p, psp)
                        if g == 3:
                            emit_l3_finalize(pr, psp)
                    if k > 0:
                        h2p = emit_l2(h1_prev, slot)
                        pend_l3.append((k - 1, slot, h2p, ps3_cur))
                while li < len(l1_units):
                    emit_l1(*l1_units[li])
                    li += 1
                while k == BC // 2 and pend_l3:
                    pr, g, h2p, psp = pend_l3.pop(0)
                    emit_l3(pr, g, h2p, psp)
                    if g == 3:
                        emit_l3_finalize(pr, psp)
                h1_prev = h1_cur

        wpool = ctx.enter_context(tc.tile_pool(name="work", bufs=1))

        # ---- gather + sigmoid fold (tanh with b3 bias) ----
        with tc.tile_pool(name="gather", bufs=1) as gpool:
            pall = gpool.tile([PP, 6 * L], F32)
            nc.sync.dma_start(pall[:], params_d[:])
            pview = pall.rearrange("p (i c4 v) -> p c4 i v", i=128, c4=4, v=6)
            U = []
            for v in range(6):
                uv = spool.tile([PP, L], F32, name=f"uparam{v}")
                nc.scalar.activation(
                    uv.rearrange("p (c4 i) -> p c4 i", c4=4),
                    pview[:, :, :, v], Act.Tanh,
                    bias=b3c[:, v : v + 1], scale=0.5,
                )
                U.append(uv)
            petT = spool.tile([PP, L], F32)
            nc.sync.dma_start(petT[:], met[0])
            tmT = spool.tile([PP, L], F32)
            nc.sync.dma_start(tmT[:], met[1])
            prT = spool.tile([PP, L], F32)
            nc.sync.dma_start(prT[:], met[2])

            # ---- coefficient precompute ----
            ph = wpool.tile([PP, L], F32, tag="w0", name="ph")
            nc.vector.tensor_scalar_mul(ph[:], prT[:], 0.5)
            wps = wpool.tile([PP, L], F32, tag="w1", name="wps")
            nc.vector.scalar_tensor_tensor(wps[:], U[0][:], -1.5, tmT[:], Op.mult, Op.subtract)
            ups = wpool.tile([PP, L], F32, tag="w2", name="ups")
            nc.scalar.activation(ups[:], wps[:], Act.Tanh, bias=cm75[:], scale=5.0)
            psnow = spool.tile([PP, L], F32)
            nc.vector.scalar_tensor_tensor(psnow[:], ups[:], 1.0, ph[:], Op.add, Op.mult)
            om = wpool.tile([PP, L], F32, tag="w3", name="om")
            nc.vector.tensor_scalar(om[:], ups[:], -1.0, 1.0, Op.mult, Op.add)
            prain = spool.tile([PP, L], F32)
            nc.vector.tensor_mul(prain[:], om[:], ph[:])
            wA = wpool.tile([PP, L], F32, tag="w4", name="wA")
            nc.vector.scalar_tensor_tensor(wA[:], U[1][:], -1.5, tmT[:], Op.mult, Op.add)
            uA = wpool.tile([PP, L], F32, tag="w5", name="uA")
            nc.scalar.activation(uA[:], wA[:], Act.Tanh, bias=cm75[:], scale=5.0)
            Ah2 = spool.tile([PP, L], F32)
            nc.vector.tensor_scalar(Ah2[:], uA[:], 0.25, 0.25, Op.mult, Op.add)
            xm = wpool.tile([PP, L], F32, tag="w6", name="xm")
            nc.vector.tensor_scalar_add(xm[:], wA[:], -1.5)
            d5 = wpool.tile([PP, L], F32, tag="w7", name="d5")
            nc.vector.tensor_scalar(d5[:], U[2][:], 2.5, 2.5, Op.mult, Op.add)
            mT = wpool.tile([PP, L], F32, tag="w8", name="mTt")
            nc.vector.tensor_mul(mT[:], d5[:], xm[:])
            AM = spool.tile([PP, L], F32)
            nc.vector.tensor_mul(AM[:], Ah2[:], mT[:])
            fT = spool.tile([PP, L], F32)
            nc.vector.tensor_scalar(fT[:], U[3][:], 0.05, 0.05, Op.mult, Op.add)
            smaxT = spool.tile([PP, L], F32)
            nc.vector.tensor_scalar(smaxT[:], U[4][:], 700.0, 800.0, Op.mult, Op.add)
            qmaxT = spool.tile([PP, L], F32)
            nc.vector.tensor_scalar(qmaxT[:], U[5][:], 20.0, 30.0, Op.mult, Op.add)
            invs = spool.tile([PP, L], F32)
            nc.vector.reciprocal(invs[:], smaxT[:])
            FQ = spool.tile([PP, L], F32)
            nc.vector.tensor_mul(FQ[:], fT[:], qmaxT[:])
            PQ = spool.tile([PP, L], F32)
            nc.vector.tensor_add(PQ[:], petT[:], qmaxT[:])
            psv = spool.tile([PP, L], F32)
            nc.gpsimd.tensor_tensor(psv[:], petT[:], invs[:], Op.mult)

        # ---- state tiles ----
        S0 = spool.tile([PP, L], F32)
        nc.vector.memset(S0[:], 0.0)
        SP0c = spool.tile([PP, 1], F32)
        nc.vector.memset(SP0c[:], 0.0)
        S1 = spool.tile([PP, L], F32)
        nc.vector.memset(S1[:], 0.0)
        SP1c = spool.tile([PP, 1], F32)
        nc.vector.memset(SP1c[:], 0.0)
        RT = spool.tile([PP, L], F32)
        dsr = spool.tile([1, PP], F32)
        nc.vector.memset(dsr[:], 0.0)

        def tw(nm):
            return wpool.tile([PP, L], F32, tag=nm, name=nm)

        with tc.tile_pool(name="ks_ps", bufs=2, space="PSUM") as kpool:

            def gate_scan(Jn, jtag):
                """cumprod of J, block-end row (masked) for the carry scan."""
                gpn = wpool.tile([PP, L], F32, tag=f"gp{jtag}", name=f"gp{jtag}")
                nc.vector.tensor_tensor_scan(
                    gpn[:], Jn[:], ones[:], 1.0, Op.mult, Op.mult
                )
                gT = kpool.tile([1, PP], F32, tag="gT", name=f"gT{jtag}")
                nc.tensor.matmul(gT[:], gpn[:, L - 1 : L], idm_s[:], start=True, stop=True)
                gmn = wpool.tile([1, PP], F32, tag=f"gm{jtag}", name=f"gm{jtag}")
                nc.vector.tensor_copy(gmn[:], gT[:])
                return gpn, gmn

            def ks_tail(dp, gmask):
                """exclusive block-carry: transpose ends, 128-long scan, back."""
                dT = kpool.tile([1, PP], F32, tag="dT", name="dT")
                nc.tensor.matmul(dT[:], dp[:, L - 1 : L], idm_s[:], start=True, stop=True)
                nc.vector.tensor_tensor_scan(
                    dsr[:, 1:PP], gmask[:, 0 : PP - 1], dT[:, 0 : PP - 1],
                    0.0, Op.mult, Op.add,
                )
                dcol = kpool.tile([PP, 1], F32, tag="dcol", name="dcol")
                nc.tensor.matmul(dcol[:], dsr[:], onecol[:], start=True, stop=True)
                dsb = wpool.tile([PP, 1], F32, tag="ksds", name="ksds")
                nc.vector.tensor_copy(dsb[:], dcol[:])
                return dsb

            # ---- snow: J/gate init at zero state ----
            sgA = tw("sg")
            nc.scalar.activation(sgA[:], AM[:], Act.Sign)
            lt3 = tw("lt3")
            nc.vector.scalar_tensor_tensor(lt3[:], sgA[:], 1.0, Ah2[:], Op.add, Op.mult)
            Jt = wpool.tile([PP, L], F32, tag="s0J", name="J0")
            nc.vector.tensor_scalar(Jt[:], lt3[:], -0.5, 1.0, Op.mult, Op.add)
            gp, gmask = gate_scan(Jt, "s0")

            # ---- snow sweeps (frozen-gate propagator, one-state-stale J) ----
            for it in range(N_S0):
                last = it == N_S0 - 1
                u = tw("u0")
                nc.scalar.activation(u[:, 1:L], S0[:, 0 : L - 1], Act.Tanh, scale=5.0)
                nc.scalar.activation(u[:, 0:1], SP0c[:], Act.Tanh, scale=5.0)
                z = tw("z")
                nc.gpsimd.tensor_tensor(z[:, 1:L], Ah2[:, 1:L], S0[:, 0 : L - 1], Op.mult)
                nc.vector.tensor_tensor(z[:, 0:1], Ah2[:, 0:1], SP0c[:], Op.mult)
                t2 = tw("t2")
                nc.gpsimd.tensor_tensor(t2[:, 1:L], S0[:, 0 : L - 1], S0[:, 1:L], Op.subtract)
                nc.vector.tensor_tensor(t2[:, 0:1], SP0c[:], S0[:, 0:1], Op.subtract)
                rr2 = tw("rr2")
                nc.gpsimd.tensor_tensor(rr2[:], psnow[:], t2[:], Op.add)
                mn2 = tw("mn")
                nc.vector.tensor_tensor(mn2[:], z[:], AM[:], Op.min)
                melt = tw("melt")
                nc.vector.scalar_tensor_tensor(melt[:], u[:], 1.0, mn2[:], Op.add, Op.mult)
                rr = tw("s1r")
                nc.vector.tensor_sub(rr[:], rr2[:], melt[:])
                dp = wpool.tile([PP, L], F32, tag="dp", name="dp")
                nc.vector.tensor_tensor_scan(dp[:], Jt[:], rr[:], 0.0, Op.mult, Op.add)
                dsb = ks_tail(dp, gmask)
                delta = tw("delta")
                nc.vector.scalar_tensor_tensor(delta[:], gp[:], dsb[:], dp[:], Op.mult, Op.add)
                nc.vector.tensor_add(S0[:], S0[:], delta[:])
                nc.vector.tensor_add(SP0c[:], SP0c[:], dsb[:])
                if not last:
                    # gate refresh from this sweep's entry state (stale by one)
                    dz = tw("dz")
                    nc.gpsimd.tensor_tensor(dz[:], mn2[:], z[:], Op.subtract)
                    sg = tw("sg")
                    nc.scalar.activation(sg[:], dz[:], Act.Sign)
                    lt3 = tw("lt3")
                    nc.vector.scalar_tensor_tensor(lt3[:], sg[:], 1.0, Ah2[:], Op.add, Op.mult)
                    jtx = tw("jtx")
                    nc.vector.scalar_tensor_tensor(jtx[:], u[:], 1.0, lt3[:], Op.add, Op.mult)
                    Jt = wpool.tile([PP, L], F32, tag="s0J", name=f"J{it + 1}")
                    nc.vector.tensor_scalar(Jt[:], jtx[:], -0.5, 1.0, Op.mult, Op.add)
                    if it % 2 == 0:
                        gp, gmask = gate_scan(Jt, "s0")

            # ---- melt from converged snow state, rain+melt forcing ----
            u = tw("u0")
            nc.scalar.activation(u[:, 1:L], S0[:, 0 : L - 1], Act.Tanh, scale=5.0)
            nc.scalar.activation(u[:, 0:1], SP0c[:], Act.Tanh, scale=5.0)
            z = tw("z")
            nc.gpsimd.tensor_tensor(z[:, 1:L], Ah2[:, 1:L], S0[:, 0 : L - 1], Op.mult)
            nc.vector.tensor_tensor(z[:, 0:1], Ah2[:, 0:1], SP0c[:], Op.mult)
            mn2 = tw("mn")
            nc.vector.tensor_tensor(mn2[:], z[:], AM[:], Op.min)
            melt = tw("melt")
            nc.vector.scalar_tensor_tensor(melt[:], u[:], 1.0, mn2[:], Op.add, Op.mult)
            nc.vector.tensor_add(RT[:], prain[:], melt[:])

            # ---- soil: J init at zero state:  J = 1 - 0.5*pet/smax ----
            Jt1 = wpool.tile([PP, L], F32, tag="s1J", name="J1_0")
            nc.vector.tensor_scalar(Jt1[:], psv[:], -0.5, 1.0, Op.mult, Op.add)
            gp1, gmask1 = gate_scan(Jt1, "s1")

            # ---- soil sweeps (clamped Newton, one-state-stale J) ----
            for it in range(N_S1):
                last = it == N_S1 - 1
                u0 = tw("u0")
                nc.scalar.activation(u0[:, 1:L], S1[:, 0 : L - 1], Act.Tanh, scale=5.0)
                nc.scalar.activation(u0[:, 0:1], SP1c[:], Act.Tanh, scale=5.0)
                dd = tw("dd")
                nc.gpsimd.tensor_tensor(dd[:, 1:L], S1[:, 0 : L - 1], smaxT[:, 1:L], Op.subtract)
                nc.vector.tensor_tensor(dd[:, 0:1], SP1c[:], smaxT[:, 0:1], Op.subtract)
                m1 = tw("m1")
                nc.gpsimd.tensor_tensor(m1[:, 1:L], psv[:, 1:L], S1[:, 0 : L - 1], Op.mult)
                nc.vector.tensor_tensor(m1[:, 0:1], psv[:, 0:1], SP1c[:], Op.mult)
                t2 = tw("t2")
                nc.gpsimd.tensor_tensor(t2[:, 1:L], S1[:, 0 : L - 1], S1[:, 1:L], Op.subtract)
                nc.vector.tensor_tensor(t2[:, 0:1], SP1c[:], S1[:, 0:1], Op.subtract)
                u1 = tw("u1")
                nc.scalar.activation(u1[:], dd[:], Act.Tanh, scale=5.0)
                ea = tw("ea")
                nc.gpsimd.tensor_tensor(ea[:], fT[:], dd[:], Op.mult)
                eac = tw("eac")
                nc.vector.tensor_scalar_min(eac[:], ea[:], 2.0)
                Ee = tw("Ee")
                nc.scalar.activation(Ee[:], eac[:], Act.Exp)
                h1 = tw("h1")
                nc.scalar.activation(h1[:], u0[:], Act.Copy, bias=0.5, scale=0.5)
                ab = tw("ab")
                nc.scalar.activation(ab[:], u1[:], Act.Copy, bias=0.5, scale=0.5)
                be = tw("be")
                nc.scalar.activation(be[:], u1[:], Act.Copy, bias=0.5, scale=-0.5)
                qE = tw("qE")
                nc.vector.tensor_mul(qE[:], qmaxT[:], Ee[:])
                m3 = tw("m3")
                nc.vector.tensor_add(m3[:], m1[:], qE[:])
                m4 = tw("m4")
                nc.vector.tensor_mul(m4[:], be[:], m3[:])
                m5 = tw("m5")
                nc.vector.tensor_mul(m5[:], ab[:], PQ[:])
                inner = tw("inner")
                nc.vector.tensor_add(inner[:], m4[:], m5[:])
                etq = tw("etq")
                nc.vector.tensor_mul(etq[:], h1[:], inner[:])
                s1a = tw("s1a")
                nc.gpsimd.tensor_tensor(s1a[:], h1[:], ab[:], Op.mult)
                qsurf = tw("qsurf")
                nc.vector.tensor_mul(qsurf[:], s1a[:], dd[:])
                g1 = tw("g1")
                nc.vector.scalar_tensor_tensor(g1[:], etq[:], -1.0, RT[:], Op.mult, Op.add)
                gg = tw("gg")
                nc.vector.tensor_sub(gg[:], g1[:], qsurf[:])
                rr = tw("s1r")
                nc.vector.tensor_add(rr[:], gg[:], t2[:])
                dp = wpool.tile([PP, L], F32, tag="dp", name="dp1")
                nc.vector.tensor_tensor_scan(dp[:], Jt1[:], rr[:], 0.0, Op.mult, Op.add)
                dsb = ks_tail(dp, gmask1)
                delta = tw("delta")
                nc.vector.scalar_tensor_tensor(delta[:], gp1[:], dsb[:], dp[:], Op.mult, Op.add)
                nc.vector.tensor_add(S1[:], S1[:], delta[:])
                nc.vector.tensor_add(SP1c[:], SP1c[:], dsb[:])
                if not last:
                    u0sq = tw("u0sq")
                    nc.gpsimd.tensor_tensor(u0sq[:], u0[:], u0[:], Op.mult)
                    h1p = tw("h1p")
                    nc.scalar.activation(h1p[:], u0sq[:], Act.Copy, bias=2.5, scale=-2.5)
                    u1sq = tw("u1sq")
                    nc.gpsimd.tensor_tensor(u1sq[:], u1[:], u1[:], Op.mult)
                    abp = tw("abp")
                    nc.scalar.activation(abp[:], u1sq[:], Act.Copy, bias=2.5, scale=-2.5)
                    d1 = tw("d1")
                    nc.vector.tensor_sub(d1[:], PQ[:], m3[:])
                    d2 = tw("d2")
                    nc.vector.tensor_mul(d2[:], abp[:], d1[:])
                    fE = tw("fE")
                    nc.gpsimd.tensor_tensor(fE[:], FQ[:], Ee[:], Op.mult)
                    d3 = tw("d3")
                    nc.vector.tensor_add(d3[:], psv[:], fE[:])
                    d4 = tw("d4")
                    nc.vector.tensor_mul(d4[:], be[:], d3[:])
                    dinner = tw("dinner")
                    nc.vector.tensor_add(dinner[:], d2[:], d4[:])
                    jA = tw("jA")
                    nc.vector.tensor_mul(jA[:], h1p[:], inner[:])
                    jB = tw("jB")
                    nc.vector.tensor_mul(jB[:], h1[:], dinner[:])
                    jq = tw("jq")
                    nc.vector.tensor_add(jq[:], jA[:], jB[:])
                    e1 = tw("e1")
                    nc.vector.tensor_mul(e1[:], h1p[:], ab[:])
                    e2 = tw("e2")
                    nc.vector.tensor_mul(e2[:], h1[:], abp[:])
                    e3 = tw("e3")
                    nc.vector.tensor_add(e3[:], e1[:], e2[:])
                    w1d = tw("w1d")
                    nc.vector.tensor_mul(w1d[:], e3[:], dd[:])
                    jq2 = tw("jq2")
                    nc.vector.tensor_add(jq2[:], w1d[:], s1a[:])
                    jtot = tw("jtot")
                    nc.vector.tensor_add(jtot[:], jq[:], jq2[:])
                    j3 = tw("j3")
                    nc.scalar.activation(j3[:], jtot[:], Act.Copy, bias=1.0, scale=-1.0)
                    Jt1 = wpool.tile([PP, L], F32, tag="s1J", name=f"J1_{it + 1}")
                    nc.vector.tensor_scalar(Jt1[:], j3[:], -1.0, 1.02, Op.max, Op.min)
                    gp1, gmask1 = gate_scan(Jt1, "s1")

        # ---- final streamflow from post-update soil state ----
        u0q = wpool.tile([PP, L], F32, tag="u0", name="u0q")
        nc.scalar.activation(u0q[:], S1[:], Act.Tanh, scale=5.0)
        dq = wpool.tile([PP, L], F32, tag="dd", name="dq")
        nc.vector.tensor_sub(dq[:], S1[:], smaxT[:])
        u1q = wpool.tile([PP, L], F32, tag="u1", name="u1q")
        nc.scalar.activation(u1q[:], dq[:], Act.Tanh, scale=5.0)
        argq = wpool.tile([PP, L], F32, tag="ea", name="argq")
        nc.gpsimd.tensor_tensor(argq[:], fT[:], dq[:], Op.mult)
        Eq = wpool.tile([PP, L], F32, tag="Ee", name="Eq")
        nc.scalar.activation(Eq[:], argq[:], Act.Exp)
        h1q = wpool.tile([PP, L], F32, tag="h1", name="h1q")
        nc.scalar.activation(h1q[:], u0q[:], Act.Copy, bias=0.5, scale=0.5)
        abq = wpool.tile([PP, L], F32, tag="ab", name="abq")
        nc.scalar.activation(abq[:], u1q[:], Act.Copy, bias=0.5, scale=0.5)
        beq = wpool.tile([PP, L], F32, tag="be", name="beq")
        nc.scalar.activation(beq[:], u1q[:], Act.Copy, bias=0.5, scale=-0.5)
        qq1 = wpool.tile([PP, L], F32, tag="m4", name="qq1")
        nc.vector.tensor_mul(qq1[:], beq[:], Eq[:])
        qq2 = wpool.tile([PP, L], F32, tag="m5", name="qq2")
        nc.vector.tensor_add(qq2[:], abq[:], qq1[:])
        qq3 = wpool.tile([PP, L], F32, tag="m3", name="qq3")
        nc.vector.tensor_mul(qq3[:], qmaxT[:], qq2[:])
        qsb = wpool.tile([PP, L], F32, tag="qE", name="qsb")
        nc.vector.tensor_mul(qsb[:], h1q[:], qq3[:])
        hab = wpool.tile([PP, L], F32, tag="s1a", name="hab")
        nc.gpsimd.tensor_tensor(hab[:], h1q[:], abq[:], Op.mult)
        qsf = wpool.tile([PP, L], F32, tag="qsurf", name="qsf")
        nc.vector.tensor_mul(qsf[:], hab[:], dq[:])
        qfin = wpool.tile([PP, L], F32, tag="gg", name="qfin")
        nc.vector.tensor_add(qfin[:], qsb[:], qsf[:])
        nc.sync.dma_start(q_out.rearrange("c (b l) -> (c b) l", l=L), qfin[:])


_CACHED = {}


def _get_module():
    if "nc" in _CACHED:
        return _CACHED["nc"]
    nc = bacc.Bacc(
        "TRN2", target_bir_lowering=False, debug=False, num_devices=NCORES
    )
    att = nc.dram_tensor("att", [BC, KA, T], BF16, kind="ExternalInput").ap()
    met = nc.dram_tensor("met", [3, PP, L], F32, kind="ExternalInput").ap()
    w1k = nc.dram_tensor("w1k", [KA, H1], BF16, kind="ExternalInput").ap()
    w2c = nc.dram_tensor("w2c", [H1, H2], BF16, kind="ExternalInput").ap()
    b2d = nc.dram_tensor("b2d", [128, 1], F32, kind="ExternalInput").ap()
    w3 = nc.dram_tensor("w3", [H2, 6], BF16, kind="ExternalInput").ap()
    b3b = nc.dram_tensor("b3b", [PP, 6], F32, kind="ExternalInput").ap()
    idm = nc.dram_tensor("idm", [PP, PP], F32, kind="ExternalInput").ap()
    q = nc.dram_tensor("q", [BC, T], F32, kind="ExternalOutput").ap()
    with tile.TileContext(nc) as tc:
        _build_kernel(tc, [q], [att, met, w1k, w2c, b2d, w3, b3b, idm])
    nc.compile()
    _CACHED["nc"] = nc
    return nc


def _bf16(a):
    return np.asarray(a, np.float32).astype(ml_dtypes.bfloat16)


def _shard_inputs(inputs):
    """Per-core input dicts: slice the catchment axis; host-side layout
    transforms + bf16 splitting only."""
    idm = _host_constants()
    xs = np.ascontiguousarray(np.asarray(inputs["inputs"], np.float32))

    w1f = np.asarray(inputs["w1"], np.float32)
    b1f = np.asarray(inputs["b1"], np.float32)
    w1hi = _bf16(w1f)
    w1lo = _bf16(w1f - w1hi.astype(np.float32))
    w1k_h = np.concatenate(
        [w1hi, w1hi, w1lo, _bf16(b1f)[None, :]], axis=0
    )  # [46, 256]
    b2f = np.asarray(inputs["b2"], np.float32)
    b3f = np.asarray(inputs["b3"], np.float32)
    common = {
        "w1k": np.ascontiguousarray(w1k_h),
        "w2c": np.ascontiguousarray(_bf16(np.asarray(inputs["w2"], np.float32))),
        "b2d": np.ascontiguousarray(
            np.concatenate([b2f, b2f]).reshape(128, 1).astype(np.float32)
        ),
        "w3": np.ascontiguousarray(_bf16(np.asarray(inputs["w3"], np.float32))),
        "b3b": np.ascontiguousarray(
            np.broadcast_to(0.5 * b3f, (PP, 6)).astype(np.float32)
        ),
        "idm": idm,
    }
    in_maps = []
    for k in range(NCORES):
        xk = xs[k * BC : (k + 1) * BC]                      # [16, T, 20]
        attf = xk[:, :, 5:20].transpose(0, 2, 1)            # [16, 15, T]
        a_hi = _bf16(attf)
        a_lo = _bf16(attf - a_hi.astype(np.float32))
        ones_row = np.ones((BC, 1, T), ml_dtypes.bfloat16)
        att = np.ascontiguousarray(
            np.concatenate([a_hi, a_lo, a_hi, ones_row], axis=1)
        )  # [16, 46, T] rows pair with w1k rows [w_hi, w_hi, w_lo, b1]
        met = np.ascontiguousarray(
            xk[:, :, 0:3].transpose(2, 0, 1).reshape(3, BC, NB, L).reshape(3, PP, L)
        )
        in_maps.append({"att": att, "met": met, **common})
    return in_maps


def kernel(**inputs):
    nc = _get_module()
    in_maps = _shard_inputs(inputs)
    res = bass_utils.run_bass_kernel_spmd(nc, in_maps, core_ids=list(range(NCORES)))
    q = np.concatenate([res.results[k]["q"] for k in range(NCORES)], axis=0)
    return q[:, :, None].astype(np.float32)


if __name__ == "__main__":
    _get_module()
    print("module built OK")


# revision 15
# speedup vs baseline: 2.1329x; 1.0483x over previous
"""Trainium2 Bass kernel for the differentiable EXP-HYDRO module.

Strategy (8 NeuronCores, data-parallel over the catchment axis):
  - Each core gets 16 catchments x 4096 timesteps.
  - Parameterization MLP on the PE in bf16: L1 uses a 3-way bf16 split of
    (attrs, w1) packed into K=46 (exact to ~2e-7, incl. a ones-row for b1),
    L2 is a single-pass bf16 matmul, L3 is bf16 tokens-on-M.  End-to-end
    param error ~7e-5, validated through the scan at rel err ~2e-3 vs the
    2e-2 gate.  tanh on ACT.
  - The sequential bucket scan is solved parallel-in-time: fixed-point
    sweeps with hardware tensor_tensor_scan.  The propagator J, its
    within-block cumprod, and the block-carry weights are evaluated one
    state behind (validated: no convergence cost), which takes the whole
    gate chain off the per-sweep critical path.  Block-boundary carries
    use a transpose -> 128-long scan -> transpose-back instead of
    Kogge-Stone rounds.  State is kept as S plus a one-column previous-
    state carry (SP views), sweeps spread across DVE / GpSimd / ACT.
  - 20 snow + 3 soil sweeps reach the MLP-error floor (~2e-3 rel).
"""

import numpy as np
from contextlib import ExitStack

import ml_dtypes

import concourse.bass as bass
import concourse.bacc as bacc
import concourse.mybir as mybir
import concourse.tile as tile
from concourse import bass_utils

F32 = mybir.dt.float32
BF16 = mybir.dt.bfloat16
Op = mybir.AluOpType
Act = mybir.ActivationFunctionType

B, T, NF = 128, 4096, 20
NCORES = 8
BC = B // NCORES          # catchments per core = 16
NB = 8                    # time blocks per catchment
L = T // NB               # 512 steps per block
PP = BC * NB              # 128 partitions
N_S0 = 18                 # snow-bucket sweeps
N_S1 = 3                  # soil-bucket sweeps
H1, H2 = 256, 64
KA = 46                   # L1 contraction rows: [a_hi, a_lo, a_hi, ones]


def _host_constants():
    # masked identity: diag 0 at block positions p%NB == NB-1 (no carry
    # across catchment boundaries in the block-carry scan)
    idm = np.diag((np.arange(PP) % NB != NB - 1).astype(np.float32)).astype(np.float32)
    return np.ascontiguousarray(idm)


def _build_kernel(tc, outs, ins):
    nc = tc.nc
    (att, met, w1k, w2c, b2d, w3, b3b, idm) = ins
    q_out = outs[0]

    with ExitStack() as ctx:
        const = ctx.enter_context(tc.tile_pool(name="const", bufs=1))
        spool = ctx.enter_context(tc.tile_pool(name="scan", bufs=1))
        dpool = ctx.enter_context(tc.tile_pool(name="dram", bufs=1, space="DRAM"))

        # ---- constants ----
        w1k_s = const.tile([KA, H1], BF16)
        nc.sync.dma_start(w1k_s[:], w1k[:])
        w2a = const.tile([128, H2], BF16)
        nc.sync.dma_start(w2a[:], w2c[0:128, :])
        w2b = const.tile([128, H2], BF16)
        nc.sync.dma_start(w2b[:], w2c[128:256, :])
        b2d_s = const.tile([128, 1], F32)
        nc.sync.dma_start(b2d_s[:], b2d[:])
        w3s = const.tile([128, 6], BF16)
        nc.sync.dma_start(w3s[0:64, :], w3[:])
        nc.sync.dma_start(w3s[64:128, :], w3[:])
        b3c = const.tile([PP, 6], F32)
        nc.sync.dma_start(b3c[:], b3b[:])
        idm_s = const.tile([PP, PP], F32)
        nc.sync.dma_start(idm_s[:], idm[:])
        onecol = const.tile([1, 1], F32)
        nc.vector.memset(onecol[:], 1.0)
        ones = const.tile([PP, L], F32)
        nc.vector.memset(ones[:], 1.0)
        cm75 = const.tile([PP, 1], F32)
        nc.vector.memset(cm75[:], -7.5)

        # ---- DRAM staging ----
        params_d = dpool.tile([PP, 6 * L], F32)

        # ---- MLP phase ----
        with tc.tile_pool(name="mlp_att", bufs=2) as apool, \
             tc.tile_pool(name="mlp_h1", bufs=1) as hpool, \
             tc.tile_pool(name="mlp_h2", bufs=2) as h2pool, \
             tc.tile_pool(name="mlp_w", bufs=2) as mwork, \
             tc.tile_pool(name="ps1", bufs=2, space="PSUM") as ps1p, \
             tc.tile_pool(name="ps2", bufs=1, space="PSUM") as ps2p, \
             tc.tile_pool(name="ps3", bufs=2, space="PSUM") as ps3p:

            def emit_l1(c, ci, half, g, att_t, h1t):
                ps1 = ps1p.tile([128, 1024], F32, tag="ps1", name=f"ps1_{c}_{half}_{g}")
                for hv in (0, 1):
                    nc.tensor.matmul(
                        ps1[:, hv * 512 : (hv + 1) * 512],
                        w1k_s[:, half * 128 : (half + 1) * 128],
                        att_t[:, g * 1024 + hv * 512 : g * 1024 + (hv + 1) * 512],
                        start=True, stop=True,
                    )
                nc.scalar.activation(h1t[:, g * 1024 : (g + 1) * 1024], ps1[:], Act.Tanh)

            def emit_l2(h1_prev, g):
                ps2 = ps2p.tile([128, 1024], F32, tag="ps2", name=f"ps2_{g}")
                # weight-major order: one LDW for w2a (4 matmuls), one for w2b
                for hv in (0, 1):
                    gsl = slice(g * 1024 + hv * 512, g * 1024 + (hv + 1) * 512)
                    osl = slice(hv * 512, (hv + 1) * 512)
                    nc.tensor.matmul(ps2[0:64, osl], w2a[:], h1_prev[(0, 0)][:, gsl],
                                     start=True, stop=False)
                    nc.tensor.matmul(ps2[64:128, osl], w2a[:], h1_prev[(1, 0)][:, gsl],
                                     start=True, stop=False)
                for hv in (0, 1):
                    gsl = slice(g * 1024 + hv * 512, g * 1024 + (hv + 1) * 512)
                    osl = slice(hv * 512, (hv + 1) * 512)
                    nc.tensor.matmul(ps2[0:64, osl], w2b[:], h1_prev[(0, 1)][:, gsl],
                                     start=False, stop=True)
                    nc.tensor.matmul(ps2[64:128, osl], w2b[:], h1_prev[(1, 1)][:, gsl],
                                     start=False, stop=True)
                h2p = h2pool.tile([128, 1024], BF16, tag="h2p", name=f"h2p_{g}")
                nc.scalar.activation(h2p[:], ps2[:], Act.Tanh, bias=b2d_s[:, 0:1])
                return h2p

            def emit_l3(pair, g, h2p, ps3_pair):
                for ci in (0, 1):
                    rows = slice(ci * 64, ci * 64 + 64)
                    for ch in range(8):
                        gl = g * 8 + ch
                        nc.tensor.matmul(
                            ps3_pair[:, ci * 192 + gl * 6 : ci * 192 + gl * 6 + 6],
                            h2p[rows, ch * 128 : (ch + 1) * 128],
                            w3s[rows, :],
                            start=True, stop=True,
                        )

            def emit_l3_finalize(pair, ps3_pair):
                for ci in (0, 1):
                    c = 2 * pair + ci
                    u3 = mwork.tile([128, 192], F32, tag=f"u3_{ci}", name=f"u3_{c}")
                    nc.scalar.activation(u3[:], ps3_pair[:, ci * 192 : (ci + 1) * 192], Act.Copy)
                    for b in range(NB):
                        p = c * NB + b
                        dst = params_d[p : p + 1, :].rearrange(
                            "o (i c4 v) -> (o i) c4 v", i=128, c4=4, v=6
                        )
                        src = u3[:, b * 24 : (b + 1) * 24].rearrange(
                            "p (c4 v) -> p c4 v", c4=4
                        )
                        nc.sync.dma_start(dst, src)

            h1_prev = None
            pend_l3 = []
            ps3_cur = None
            for k in range(BC // 2 + 1):
                l1_units = []
                h1_cur = None
                if k > 0:
                    ps3_cur = ps3p.tile([128, 384], F32, tag="ps3",
                                        name=f"ps3p_{k - 1}")
                if k < BC // 2:
                    att_ts = {}
                    for ci in (0, 1):
                        c = 2 * k + ci
                        att_t = apool.tile([KA, T], BF16, tag=f"att{ci}",
                                           name=f"att_{c}")
                        nc.sync.dma_start(att_t[:], att[c])
                        att_ts[ci] = att_t
                    h1_cur = {}
                    for ci in (0, 1):
                        for half in (0, 1):
                            h1_cur[(ci, half)] = hpool.tile(
                                [128, T], BF16, tag=f"h1_{k % 2}_{ci}_{half}",
                                name=f"h1_{k}_{ci}_{half}",
                            )
                    # (ci, half)-major so consecutive L1 matmuls share lhsT
                    for ci in (0, 1):
                        for half in (0, 1):
                            for g in range(4):
                                l1_units.append(
                                    (2 * k + ci, ci, half, g, att_ts[ci],
                                     h1_cur[(ci, half)])
                                )
                li = 0
                for slot in range(4):
                    for _ in range(4):
                        if li < len(l1_units):
                            emit_l1(*l1_units[li])
                            li += 1
                    if pend_l3:
                        pr, g, h2p, psp = pend_l3.pop(0)
                        emit_l3(pr, g, h2p, psp)
                        if g == 3:
                            emit_l3_finalize(pr, psp)
                    if k > 0:
                        h2p = emit_l2(h1_prev, slot)
                        pend_l3.append((k - 1, slot, h2p, ps3_cur))
                while li < len(l1_units):
                    emit_l1(*l1_units[li])
                    li += 1
                while k == BC // 2 and pend_l3:
                    pr, g, h2p, psp = pend_l3.pop(0)
                    emit_l3(pr, g, h2p, psp)
                    if g == 3:
                        emit_l3_finalize(pr, psp)
                h1_prev = h1_cur

        wpool = ctx.enter_context(tc.tile_pool(name="work", bufs=1))

        # ---- gather + sigmoid fold (tanh with b3 bias) ----
        with tc.tile_pool(name="gather", bufs=1) as gpool:
            pall = gpool.tile([PP, 6 * L], F32)
            nc.sync.dma_start(pall[:], params_d[:])
            pview = pall.rearrange("p (i c4 v) -> p c4 i v", i=128, c4=4, v=6)
            U = []
            for v in range(6):
                uv = spool.tile([PP, L], F32, name=f"uparam{v}")
                nc.scalar.activation(
                    uv.rearrange("p (c4 i) -> p c4 i", c4=4),
                    pview[:, :, :, v], Act.Tanh,
                    bias=b3c[:, v : v + 1], scale=0.5,
                )
                U.append(uv)
            petT = spool.tile([PP, L], F32)
            nc.sync.dma_start(petT[:], met[0])
            tmT = spool.tile([PP, L], F32)
            nc.sync.dma_start(tmT[:], met[1])
            prT = spool.tile([PP, L], F32)
            nc.sync.dma_start(prT[:], met[2])

            # ---- coefficient precompute ----
            ph = wpool.tile([PP, L], F32, tag="w0", name="ph")
            nc.vector.tensor_scalar_mul(ph[:], prT[:], 0.5)
            wps = wpool.tile([PP, L], F32, tag="w1", name="wps")
            nc.vector.scalar_tensor_tensor(wps[:], U[0][:], -1.5, tmT[:], Op.mult, Op.subtract)
            ups = wpool.tile([PP, L], F32, tag="w2", name="ups")
            nc.scalar.activation(ups[:], wps[:], Act.Tanh, bias=cm75[:], scale=5.0)
            psnow = spool.tile([PP, L], F32)
            nc.vector.scalar_tensor_tensor(psnow[:], ups[:], 1.0, ph[:], Op.add, Op.mult)
            om = wpool.tile([PP, L], F32, tag="w3", name="om")
            nc.vector.tensor_scalar(om[:], ups[:], -1.0, 1.0, Op.mult, Op.add)
            prain = spool.tile([PP, L], F32)
            nc.vector.tensor_mul(prain[:], om[:], ph[:])
            wA = wpool.tile([PP, L], F32, tag="w4", name="wA")
            nc.vector.scalar_tensor_tensor(wA[:], U[1][:], -1.5, tmT[:], Op.mult, Op.add)
            uA = wpool.tile([PP, L], F32, tag="w5", name="uA")
            nc.scalar.activation(uA[:], wA[:], Act.Tanh, bias=cm75[:], scale=5.0)
            Ah2 = spool.tile([PP, L], F32)
            nc.vector.tensor_scalar(Ah2[:], uA[:], 0.25, 0.25, Op.mult, Op.add)
            xm = wpool.tile([PP, L], F32, tag="w6", name="xm")
            nc.vector.tensor_scalar_add(xm[:], wA[:], -1.5)
            d5 = wpool.tile([PP, L], F32, tag="w7", name="d5")
            nc.vector.tensor_scalar(d5[:], U[2][:], 2.5, 2.5, Op.mult, Op.add)
            mT = wpool.tile([PP, L], F32, tag="w8", name="mTt")
            nc.vector.tensor_mul(mT[:], d5[:], xm[:])
            AM = spool.tile([PP, L], F32)
            nc.vector.tensor_mul(AM[:], Ah2[:], mT[:])
            fT = spool.tile([PP, L], F32)
            nc.vector.tensor_scalar(fT[:], U[3][:], 0.05, 0.05, Op.mult, Op.add)
            smaxT = spool.tile([PP, L], F32)
            nc.vector.tensor_scalar(smaxT[:], U[4][:], 700.0, 800.0, Op.mult, Op.add)
            qmaxT = spool.tile([PP, L], F32)
            nc.vector.tensor_scalar(qmaxT[:], U[5][:], 20.0, 30.0, Op.mult, Op.add)
            invs = spool.tile([PP, L], F32)
            nc.vector.reciprocal(invs[:], smaxT[:])
            FQ = spool.tile([PP, L], F32)
            nc.vector.tensor_mul(FQ[:], fT[:], qmaxT[:])
            PQ = spool.tile([PP, L], F32)
            nc.vector.tensor_add(PQ[:], petT[:], qmaxT[:])
            psv = spool.tile([PP, L], F32)
            nc.gpsimd.tensor_tensor(psv[:], petT[:], invs[:], Op.mult)

        # ---- state tiles ----
        S0 = spool.tile([PP, L], F32)
        nc.vector.memset(S0[:], 0.0)
        SP0c = spool.tile([PP, 1], F32)
        nc.vector.memset(SP0c[:], 0.0)
        S1 = spool.tile([PP, L], F32)
        nc.vector.memset(S1[:], 0.0)
        SP1c = spool.tile([PP, 1], F32)
        nc.vector.memset(SP1c[:], 0.0)
        RT = spool.tile([PP, L], F32)
        dsr = spool.tile([1, PP], F32)
        nc.vector.memset(dsr[:], 0.0)

        def tw(nm):
            return wpool.tile([PP, L], F32, tag=nm, name=nm)

        with tc.tile_pool(name="ks_ps", bufs=2, space="PSUM") as kpool:

            def gate_scan(Jn, jtag):
                """cumprod of J, block-end row (masked) for the carry scan."""
                gpn = wpool.tile([PP, L], F32, tag=f"gp{jtag}", name=f"gp{jtag}")
                nc.vector.tensor_tensor_scan(
                    gpn[:], Jn[:], ones[:], 1.0, Op.mult, Op.mult
                )
                gT = kpool.tile([1, PP], F32, tag="gT", name=f"gT{jtag}")
                nc.tensor.matmul(gT[:], gpn[:, L - 1 : L], idm_s[:], start=True, stop=True)
                gmn = wpool.tile([1, PP], F32, tag=f"gm{jtag}", name=f"gm{jtag}")
                nc.vector.tensor_copy(gmn[:], gT[:])
                return gpn, gmn

            def ks_tail(dp, gmask):
                """exclusive block-carry: transpose ends, 128-long scan, back."""
                dT = kpool.tile([1, PP], F32, tag="dT", name="dT")
                nc.tensor.matmul(dT[:], dp[:, L - 1 : L], idm_s[:], start=True, stop=True)
                nc.vector.tensor_tensor_scan(
                    dsr[:, 1:PP], gmask[:, 0 : PP - 1], dT[:, 0 : PP - 1],
                    0.0, Op.mult, Op.add,
                )
                dcol = kpool.tile([PP, 1], F32, tag="dcol", name="dcol")
                nc.tensor.matmul(dcol[:], dsr[:], onecol[:], start=True, stop=True)
                dsb = wpool.tile([PP, 1], F32, tag="ksds", name="ksds")
                nc.vector.tensor_copy(dsb[:], dcol[:])
                return dsb

            # ---- snow: J/gate init at zero state ----
            sgA = tw("sg")
            nc.scalar.activation(sgA[:], AM[:], Act.Sign)
            lt3 = tw("lt3")
            nc.vector.scalar_tensor_tensor(lt3[:], sgA[:], 1.0, Ah2[:], Op.add, Op.mult)
            Jt = wpool.tile([PP, L], F32, tag="s0J", name="J0")
            nc.vector.tensor_scalar(Jt[:], lt3[:], -0.5, 1.0, Op.mult, Op.add)
            gp, gmask = gate_scan(Jt, "s0")

            # ---- snow sweeps (frozen-gate propagator, one-state-stale J) ----
            for it in range(N_S0):
                last = it == N_S0 - 1
                u = tw("u0")
                nc.scalar.activation(u[:, 1:L], S0[:, 0 : L - 1], Act.Tanh, scale=5.0)
                nc.scalar.activation(u[:, 0:1], SP0c[:], Act.Tanh, scale=5.0)
                z = tw("z")
                nc.vector.tensor_tensor(z[:, 1:L], Ah2[:, 1:L], S0[:, 0 : L - 1], Op.mult)
                nc.vector.tensor_tensor(z[:, 0:1], Ah2[:, 0:1], SP0c[:], Op.mult)
                t2 = tw("t2")
                nc.gpsimd.tensor_tensor(t2[:, 1:L], S0[:, 0 : L - 1], S0[:, 1:L], Op.subtract)
                nc.vector.tensor_tensor(t2[:, 0:1], SP0c[:], S0[:, 0:1], Op.subtract)
                rr2 = tw("rr2")
                nc.gpsimd.tensor_tensor(rr2[:], psnow[:], t2[:], Op.add)
                mn2 = tw("mn")
                nc.vector.tensor_tensor(mn2[:], z[:], AM[:], Op.min)
                melt = tw("melt")
                nc.vector.scalar_tensor_tensor(melt[:], u[:], 1.0, mn2[:], Op.add, Op.mult)
                rr = tw("s1r")
                nc.vector.tensor_sub(rr[:], rr2[:], melt[:])
                dp = wpool.tile([PP, L], F32, tag="dp", name="dp")
                nc.vector.tensor_tensor_scan(dp[:], Jt[:], rr[:], 0.0, Op.mult, Op.add)
                dsb = ks_tail(dp, gmask)
                delta = tw("delta")
                nc.vector.scalar_tensor_tensor(delta[:], gp[:], dsb[:], dp[:], Op.mult, Op.add)
                nc.vector.tensor_add(S0[:], S0[:], delta[:])
                nc.vector.tensor_add(SP0c[:], SP0c[:], dsb[:])
                if not last:
                    # gate refresh from this sweep's entry state (stale by one)
                    dz = tw("dz")
                    nc.gpsimd.tensor_tensor(dz[:], mn2[:], z[:], Op.subtract)
                    sg = tw("sg")
                    nc.scalar.activation(sg[:], dz[:], Act.Sign)
                    lt3 = tw("lt3")
                    nc.vector.scalar_tensor_tensor(lt3[:], sg[:], 1.0, Ah2[:], Op.add, Op.mult)
                    jtx = tw("jtx")
                    nc.vector.scalar_tensor_tensor(jtx[:], u[:], 1.0, lt3[:], Op.add, Op.mult)
                    Jt = wpool.tile([PP, L], F32, tag="s0J", name=f"J{it + 1}")
                    nc.vector.tensor_scalar(Jt[:], jtx[:], -0.5, 1.0, Op.mult, Op.add)
                    if it % 2 == 0:
                        gp, gmask = gate_scan(Jt, "s0")

            # ---- melt from converged snow state, rain+melt forcing ----
            u = tw("u0")
            nc.scalar.activation(u[:, 1:L], S0[:, 0 : L - 1], Act.Tanh, scale=5.0)
            nc.scalar.activation(u[:, 0:1], SP0c[:], Act.Tanh, scale=5.0)
            z = tw("z")
            nc.vector.tensor_tensor(z[:, 1:L], Ah2[:, 1:L], S0[:, 0 : L - 1], Op.mult)
            nc.vector.tensor_tensor(z[:, 0:1], Ah2[:, 0:1], SP0c[:], Op.mult)
            mn2 = tw("mn")
            nc.vector.tensor_tensor(mn2[:], z[:], AM[:], Op.min)
            melt = tw("melt")
            nc.vector.scalar_tensor_tensor(melt[:], u[:], 1.0, mn2[:], Op.add, Op.mult)
            nc.vector.tensor_add(RT[:], prain[:], melt[:])

            # ---- soil: J init at zero state:  J = 1 - 0.5*pet/smax ----
            Jt1 = wpool.tile([PP, L], F32, tag="s1J", name="J1_0")
            nc.vector.tensor_scalar(Jt1[:], psv[:], -0.5, 1.0, Op.mult, Op.add)
            gp1, gmask1 = gate_scan(Jt1, "s1")

            # ---- soil sweeps (clamped Newton, one-state-stale J) ----
            for it in range(N_S1):
                last = it == N_S1 - 1
                u0 = tw("u0")
                nc.scalar.activation(u0[:, 1:L], S1[:, 0 : L - 1], Act.Tanh, scale=5.0)
                nc.scalar.activation(u0[:, 0:1], SP1c[:], Act.Tanh, scale=5.0)
                dd = tw("dd")
                nc.vector.tensor_tensor(dd[:, 1:L], S1[:, 0 : L - 1], smaxT[:, 1:L], Op.subtract)
                nc.vector.tensor_tensor(dd[:, 0:1], SP1c[:], smaxT[:, 0:1], Op.subtract)
                m1 = tw("m1")
                nc.gpsimd.tensor_tensor(m1[:, 1:L], psv[:, 1:L], S1[:, 0 : L - 1], Op.mult)
                nc.vector.tensor_tensor(m1[:, 0:1], psv[:, 0:1], SP1c[:], Op.mult)
                t2 = tw("t2")
                nc.gpsimd.tensor_tensor(t2[:, 1:L], S1[:, 0 : L - 1], S1[:, 1:L], Op.subtract)
                nc.vector.tensor_tensor(t2[:, 0:1], SP1c[:], S1[:, 0:1], Op.subtract)
                u1 = tw("u1")
                nc.scalar.activation(u1[:], dd[:], Act.Tanh, scale=5.0)
                ea = tw("ea")
                nc.gpsimd.tensor_tensor(ea[:], fT[:], dd[:], Op.mult)
                eac = tw("eac")
                nc.vector.tensor_scalar_min(eac[:], ea[:], 2.0)
                Ee = tw("Ee")
                nc.scalar.activation(Ee[:], eac[:], Act.Exp)
                h1 = tw("h1")
                nc.scalar.activation(h1[:], u0[:], Act.Copy, bias=0.5, scale=0.5)
                ab = tw("ab")
                nc.scalar.activation(ab[:], u1[:], Act.Copy, bias=0.5, scale=0.5)
                be = tw("be")
                nc.scalar.activation(be[:], u1[:], Act.Copy, bias=0.5, scale=-0.5)
                qE = tw("qE")
                nc.vector.tensor_mul(qE[:], qmaxT[:], Ee[:])
                m3 = tw("m3")
                nc.vector.tensor_add(m3[:], m1[:], qE[:])
                m4 = tw("m4")
                nc.vector.tensor_mul(m4[:], be[:], m3[:])
                m5 = tw("m5")
                nc.vector.tensor_mul(m5[:], ab[:], PQ[:])
                inner = tw("inner")
                nc.vector.tensor_add(inner[:], m4[:], m5[:])
                etq = tw("etq")
                nc.vector.tensor_mul(etq[:], h1[:], inner[:])
                s1a = tw("s1a")
                nc.gpsimd.tensor_tensor(s1a[:], h1[:], ab[:], Op.mult)
                qsurf = tw("qsurf")
                nc.vector.tensor_mul(qsurf[:], s1a[:], dd[:])
                g1 = tw("g1")
                nc.vector.scalar_tensor_tensor(g1[:], etq[:], -1.0, RT[:], Op.mult, Op.add)
                gg = tw("gg")
                nc.vector.tensor_sub(gg[:], g1[:], qsurf[:])
                rr = tw("s1r")
                nc.vector.tensor_add(rr[:], gg[:], t2[:])
                dp = wpool.tile([PP, L], F32, tag="dp", name="dp1")
                nc.vector.tensor_tensor_scan(dp[:], Jt1[:], rr[:], 0.0, Op.mult, Op.add)
                dsb = ks_tail(dp, gmask1)
                delta = tw("delta")
                nc.vector.scalar_tensor_tensor(delta[:], gp1[:], dsb[:], dp[:], Op.mult, Op.add)
                nc.vector.tensor_add(S1[:], S1[:], delta[:])
                nc.vector.tensor_add(SP1c[:], SP1c[:], dsb[:])
                if not last:
                    u0sq = tw("u0sq")
                    nc.gpsimd.tensor_tensor(u0sq[:], u0[:], u0[:], Op.mult)
                    h1p = tw("h1p")
                    nc.scalar.activation(h1p[:], u0sq[:], Act.Copy, bias=2.5, scale=-2.5)
                    u1sq = tw("u1sq")
                    nc.gpsimd.tensor_tensor(u1sq[:], u1[:], u1[:], Op.mult)
                    abp = tw("abp")
                    nc.scalar.activation(abp[:], u1sq[:], Act.Copy, bias=2.5, scale=-2.5)
                    d1 = tw("d1")
                    nc.vector.tensor_sub(d1[:], PQ[:], m3[:])
                    d2 = tw("d2")
                    nc.vector.tensor_mul(d2[:], abp[:], d1[:])
                    fE = tw("fE")
                    nc.gpsimd.tensor_tensor(fE[:], FQ[:], Ee[:], Op.mult)
                    d3 = tw("d3")
                    nc.vector.tensor_add(d3[:], psv[:], fE[:])
                    d4 = tw("d4")
                    nc.vector.tensor_mul(d4[:], be[:], d3[:])
                    dinner = tw("dinner")
                    nc.vector.tensor_add(dinner[:], d2[:], d4[:])
                    jA = tw("jA")
                    nc.vector.tensor_mul(jA[:], h1p[:], inner[:])
                    jB = tw("jB")
                    nc.vector.tensor_mul(jB[:], h1[:], dinner[:])
                    jq = tw("jq")
                    nc.vector.tensor_add(jq[:], jA[:], jB[:])
                    e1 = tw("e1")
                    nc.vector.tensor_mul(e1[:], h1p[:], ab[:])
                    e2 = tw("e2")
                    nc.vector.tensor_mul(e2[:], h1[:], abp[:])
                    e3 = tw("e3")
                    nc.vector.tensor_add(e3[:], e1[:], e2[:])
                    w1d = tw("w1d")
                    nc.vector.tensor_mul(w1d[:], e3[:], dd[:])
                    jq2 = tw("jq2")
                    nc.vector.tensor_add(jq2[:], w1d[:], s1a[:])
                    jtot = tw("jtot")
                    nc.vector.tensor_add(jtot[:], jq[:], jq2[:])
                    j3 = tw("j3")
                    nc.scalar.activation(j3[:], jtot[:], Act.Copy, bias=1.0, scale=-1.0)
                    Jt1 = wpool.tile([PP, L], F32, tag="s1J", name=f"J1_{it + 1}")
                    nc.vector.tensor_scalar(Jt1[:], j3[:], -1.0, 1.02, Op.max, Op.min)
                    gp1, gmask1 = gate_scan(Jt1, "s1")

        # ---- final streamflow from post-update soil state ----
        u0q = wpool.tile([PP, L], F32, tag="u0", name="u0q")
        nc.scalar.activation(u0q[:], S1[:], Act.Tanh, scale=5.0)
        dq = wpool.tile([PP, L], F32, tag="dd", name="dq")
        nc.vector.tensor_sub(dq[:], S1[:], smaxT[:])
        u1q = wpool.tile([PP, L], F32, tag="u1", name="u1q")
        nc.scalar.activation(u1q[:], dq[:], Act.Tanh, scale=5.0)
        argq = wpool.tile([PP, L], F32, tag="ea", name="argq")
        nc.gpsimd.tensor_tensor(argq[:], fT[:], dq[:], Op.mult)
        Eq = wpool.tile([PP, L], F32, tag="Ee", name="Eq")
        nc.scalar.activation(Eq[:], argq[:], Act.Exp)
        h1q = wpool.tile([PP, L], F32, tag="h1", name="h1q")
        nc.scalar.activation(h1q[:], u0q[:], Act.Copy, bias=0.5, scale=0.5)
        abq = wpool.tile([PP, L], F32, tag="ab", name="abq")
        nc.scalar.activation(abq[:], u1q[:], Act.Copy, bias=0.5, scale=0.5)
        beq = wpool.tile([PP, L], F32, tag="be", name="beq")
        nc.scalar.activation(beq[:], u1q[:], Act.Copy, bias=0.5, scale=-0.5)
        qq1 = wpool.tile([PP, L], F32, tag="m4", name="qq1")
        nc.vector.tensor_mul(qq1[:], beq[:], Eq[:])
        qq2 = wpool.tile([PP, L], F32, tag="m5", name="qq2")
        nc.vector.tensor_add(qq2[:], abq[:], qq1[:])
        qq3 = wpool.tile([PP, L], F32, tag="m3", name="qq3")
        nc.vector.tensor_mul(qq3[:], qmaxT[:], qq2[:])
        qsb = wpool.tile([PP, L], F32, tag="qE", name="qsb")
        nc.vector.tensor_mul(qsb[:], h1q[:], qq3[:])
        hab = wpool.tile([PP, L], F32, tag="s1a", name="hab")
        nc.gpsimd.tensor_tensor(hab[:], h1q[:], abq[:], Op.mult)
        qsf = wpool.tile([PP, L], F32, tag="qsurf", name="qsf")
        nc.vector.tensor_mul(qsf[:], hab[:], dq[:])
        qfin = wpool.tile([PP, L], F32, tag="gg", name="qfin")
        nc.vector.tensor_add(qfin[:], qsb[:], qsf[:])
        nc.sync.dma_start(q_out.rearrange("c (b l) -> (c b) l", l=L), qfin[:])


_CACHED = {}


def _get_module():
    if "nc" in _CACHED:
        return _CACHED["nc"]
    nc = bacc.Bacc(
        "TRN2", target_bir_lowering=False, debug=False, num_devices=NCORES
    )
    att = nc.dram_tensor("att", [BC, KA, T], BF16, kind="ExternalInput").ap()
    met = nc.dram_tensor("met", [3, PP, L], F32, kind="ExternalInput").ap()
    w1k = nc.dram_tensor("w1k", [KA, H1], BF16, kind="ExternalInput").ap()
    w2c = nc.dram_tensor("w2c", [H1, H2], BF16, kind="ExternalInput").ap()
    b2d = nc.dram_tensor("b2d", [128, 1], F32, kind="ExternalInput").ap()
    w3 = nc.dram_tensor("w3", [H2, 6], BF16, kind="ExternalInput").ap()
    b3b = nc.dram_tensor("b3b", [PP, 6], F32, kind="ExternalInput").ap()
    idm = nc.dram_tensor("idm", [PP, PP], F32, kind="ExternalInput").ap()
    q = nc.dram_tensor("q", [BC, T], F32, kind="ExternalOutput").ap()
    with tile.TileContext(nc) as tc:
        _build_kernel(tc, [q], [att, met, w1k, w2c, b2d, w3, b3b, idm])
    nc.compile()
    _CACHED["nc"] = nc
    return nc


def _bf16(a):
    return np.asarray(a, np.float32).astype(ml_dtypes.bfloat16)


def _shard_inputs(inputs):
    """Per-core input dicts: slice the catchment axis; host-side layout
    transforms + bf16 splitting only."""
    idm = _host_constants()
    xs = np.ascontiguousarray(np.asarray(inputs["inputs"], np.float32))

    w1f = np.asarray(inputs["w1"], np.float32)
    b1f = np.asarray(inputs["b1"], np.float32)
    w1hi = _bf16(w1f)
    w1lo = _bf16(w1f - w1hi.astype(np.float32))
    w1k_h = np.concatenate(
        [w1hi, w1hi, w1lo, _bf16(b1f)[None, :]], axis=0
    )  # [46, 256]
    b2f = np.asarray(inputs["b2"], np.float32)
    b3f = np.asarray(inputs["b3"], np.float32)
    common = {
        "w1k": np.ascontiguousarray(w1k_h),
        "w2c": np.ascontiguousarray(_bf16(np.asarray(inputs["w2"], np.float32))),
        "b2d": np.ascontiguousarray(
            np.concatenate([b2f, b2f]).reshape(128, 1).astype(np.float32)
        ),
        "w3": np.ascontiguousarray(_bf16(np.asarray(inputs["w3"], np.float32))),
        "b3b": np.ascontiguousarray(
            np.broadcast_to(0.5 * b3f, (PP, 6)).astype(np.float32)
        ),
        "idm": idm,
    }
    in_maps = []
    for k in range(NCORES):
        xk = xs[k * BC : (k + 1) * BC]                      # [16, T, 20]
        attf = xk[:, :, 5:20].transpose(0, 2, 1)            # [16, 15, T]
        a_hi = _bf16(attf)
        a_lo = _bf16(attf - a_hi.astype(np.float32))
        ones_row = np.ones((BC, 1, T), ml_dtypes.bfloat16)
        att = np.ascontiguousarray(
            np.concatenate([a_hi, a_lo, a_hi, ones_row], axis=1)
        )  # [16, 46, T] rows pair with w1k rows [w_hi, w_hi, w_lo, b1]
        met = np.ascontiguousarray(
            xk[:, :, 0:3].transpose(2, 0, 1).reshape(3, BC, NB, L).reshape(3, PP, L)
        )
        in_maps.append({"att": att, "met": met, **common})
    return in_maps


def kernel(**inputs):
    nc = _get_module()
    in_maps = _shard_inputs(inputs)
    res = bass_utils.run_bass_kernel_spmd(nc, in_maps, core_ids=list(range(NCORES)))
    q = np.concatenate([res.results[k]["q"] for k in range(NCORES)], axis=0)
    return q[:, :, None].astype(np.float32)


if __name__ == "__main__":
    _get_module()
    print("module built OK")
